# revision 1
# baseline (speedup 1.0000x reference)
"""ChirpLinker Trainium2 Bass kernel (optimized).

Per core: 2 batch elements, fused into [128, NCH, B_LOC, K] scan tiles
(partition = window-within-chunk, free = chunk x batch x k).

Key optimizations over the DMA-shift baseline:
- Matching (KxK mutual best match) balanced across DVE / Pool(gpsimd) /
  Activation engines; selection phase in int16 (2x DVE mode).
- Window shifts via PE matmuls with shifted-identity stationaries
  (float32r, 1 cycle/row) instead of DRAM round-trips.
- Pointers stored biased (+8192) flat in-batch (b*64+k+1), null = 0, so
  scatter indices are a single add against a precomputed offset vector;
  no mask/补偿 ops.
- f32 payloads scattered via u16-pair views (single scatter, no
  split16/join16 packing).
"""
import numpy as np

import concourse.bass as bass
import concourse.bacc as bacc_mod
import concourse.mybir as mybir
from concourse.bass_utils import run_bass_kernel_spmd
from concourse.tile import TileContext

F32 = mybir.dt.float32
F32R = mybir.dt.float32r
I16 = mybir.dt.int16
U16 = mybir.dt.uint16
ALU = mybir.AluOpType
AX = mybir.AxisListType
ACT = mybir.ActivationFunctionType

PI = float(np.float32(np.pi))
INV2PI = float(np.float32(1.0 / (2.0 * np.pi)))
C2 = INV2PI * INV2PI

B_LOC = 2
W = 512
K = 64
NCH = 4
NSTEP = 5
FB = NCH * B_LOC * K          # 512 flat scan free size
BIAS = 8192.0
THR = 8000.0                  # biased-null threshold

_CACHE = {}


def bc_last(ap, n=K):
    return ap.to_broadcast(list(ap.shape) + [n])


def bc_mid(ap2d, n=K):
    s = ap2d.shape
    return ap2d.rearrange("p (o k) -> p o k", o=1).to_broadcast([s[0], n, s[1]])


def build_kernel():
    nc = bacc_mod.Bacc("TRN2", target_bir_lowering=False)
    tok_d = nc.declare_dram_parameter("tokens", [B_LOC, W, K, 9], F32,
                                      isOutput=False)
    out_d = nc.declare_dram_parameter("out", [B_LOC, W, K, 10], F32,
                                      isOutput=True)
    cnt_d = nc.declare_dram_parameter("counts", [1, B_LOC], F32, isOutput=True)

    with TileContext(nc) as tc:
        with (
            tc.tile_pool(name="const", bufs=1) as cpool,
            tc.tile_pool(name="kk", bufs=1) as kkpool,
            tc.tile_pool(name="tok", bufs=1) as tokpool,
            tc.tile_pool(name="sc", bufs=1) as spool,
            tc.tile_pool(name="ps", bufs=1, space="PSUM") as pspool,
        ):
            # ---------------- constants ----------------
            iota_rev = cpool.tile([128, K], I16)
            nc.gpsimd.iota(iota_rev[:], pattern=[[-1, K]], base=K,
                           channel_multiplier=0)
            # idx offset: c*128 - 8193  (idx = ptr_biased + offs)
            iota_inv = cpool.tile([128, FB], I16)
            nc.gpsimd.iota(iota_inv[:], pattern=[[0, NCH], [K, B_LOC], [1, K]],
                           base=int(BIAS) + 1, channel_multiplier=0)
            offs_c = cpool.tile([128, FB], I16)
            nc.gpsimd.iota(offs_c[:], pattern=[[B_LOC * K, NCH], [0, B_LOC],
                                               [0, K]],
                           base=-(int(BIAS) + 1), channel_multiplier=0)
            # u16-pair idx offset: 2*(c*128 - 8193) + e
            offs_p2 = cpool.tile([128, FB, 2], I16)
            nc.gpsimd.iota(offs_p2.rearrange("p f e -> p (f e)"),
                           pattern=[[2 * B_LOC * K, NCH], [0, B_LOC], [0, K],
                                    [1, 2]],
                           base=-2 * (int(BIAS) + 1), channel_multiplier=0)
            ones_i16 = cpool.tile([128, FB], I16)
            nc.vector.memset(ones_i16[:], 1)
            zer64 = cpool.tile([128, K], F32)
            nc.vector.memset(zer64[:], 0)
            tri_i = cpool.tile([128, 128], I16)
            nc.gpsimd.iota(tri_i[:], pattern=[[1, 128]], base=0,
                           channel_multiplier=-1)
            tri = cpool.tile([128, 128], F32)
            nc.vector.tensor_scalar(tri[:], tri_i[:], 0.0, None, ALU.is_gt)
            ones128 = cpool.tile([128, 128], F32)
            nc.vector.memset(ones128[:], 1.0)
            m127_i = cpool.tile([128, K], I16, tag="m127i")
            nc.gpsimd.iota(m127_i[:], pattern=[[0, K]], base=0,
                           channel_multiplier=1)
            m127i = cpool.tile([128, K], I16, tag="m127")
            nc.vector.tensor_scalar(m127i[:], m127_i[:], 127.0, None,
                                    ALU.is_lt)
            iota_up = cpool.tile([128, K], I16)
            nc.gpsimd.iota(iota_up[:], pattern=[[1, K]], base=0,
                           channel_multiplier=0)

            # shifted-identity stationaries for PE window shifts
            smat = {}
            for d in (1, 2, 4, 8, 16):
                for sgn in (1, -1):
                    dd = d * sgn
                    lo_i = cpool.tile([128, 128], I16, tag=f"slo_i")
                    nc.gpsimd.iota(lo_i[:], pattern=[[-1, 128]], base=-dd,
                                   channel_multiplier=1)
                    lo = cpool.tile([128, 128], F32, tag=f"slo{dd}")
                    nc.vector.tensor_scalar(lo[:], lo_i[:], 0.0, None,
                                            ALU.is_equal)
                    hi_i = cpool.tile([128, 128], I16, tag=f"shi_i")
                    base = (128 - dd) if dd > 0 else (-dd - 128)
                    nc.gpsimd.iota(hi_i[:], pattern=[[-1, 128]], base=base,
                                   channel_multiplier=1)
                    hi = cpool.tile([128, 128], F32, tag=f"shi{dd}")
                    nc.vector.tensor_scalar(hi[:], hi_i[:], 0.0, None,
                                            ALU.is_equal)
                    smat[dd] = (lo, hi)

            def pe_shift(x_tile, dd, tag):
                """x shifted by dd windows -> PSUM tile [128, NCH, B, K].
                Beyond-range rows become 0 (= null for biased pointers)."""
                lo, hi = smat[dd]
                ps = pspool.tile([128, NCH, B_LOC, K], F32, tag=tag)
                pf = ps.rearrange("p c b k -> p (c b k)")
                xf = x_tile.rearrange("p c b k -> p (c b k)")
                cb = B_LOC * K
                nc.tensor.matmul(pf[:, :], lo[:],
                                 xf[:, :],
                                 start=True, stop=False, skip_group_check=True)
                if dd > 0:
                    nc.tensor.matmul(pf[:, 0:(NCH - 1) * cb],
                                     hi[:],
                                     xf[:, cb:NCH * cb],
                                     start=False, stop=True,
                                     skip_group_check=True)
                else:
                    nc.tensor.matmul(pf[:, cb:NCH * cb],
                                     hi[:],
                                     xf[:, 0:(NCH - 1) * cb],
                                     start=False, stop=True,
                                     skip_group_check=True)
                return ps

            def sc16(data_i16, idx_i16, tag):
                out = spool.tile([128, NCH, B_LOC, K], I16, tag=tag)
                nc.gpsimd.local_scatter(
                    out.rearrange("p c b k -> p (c b k)"),
                    data_i16.rearrange("p c b k -> p (c b k)")
                    if len(data_i16.shape) == 4 else data_i16[:],
                    idx_i16[:] if len(idx_i16.shape) == 2
                    else idx_i16.rearrange("p c b k -> p (c b k)"),
                    channels=128, num_elems=FB, num_idxs=FB)
                return out

            def sc32(data_f32, idxp_i16, tag):
                """scatter f32 payload via u16 pairs; zero-filled slots."""
                out = spool.tile([128, NCH, B_LOC, K], F32, tag=tag)
                nc.gpsimd.local_scatter(
                    out.bitcast(U16).rearrange("p c b k -> p (c b k)"),
                    data_f32.bitcast(U16).rearrange("p c b k -> p (c b k)")
                    if len(data_f32.shape) == 4
                    else data_f32.bitcast(U16)[:],
                    idxp_i16.rearrange("p f e -> p (f e)"),
                    channels=128, num_elems=2 * FB, num_idxs=2 * FB)
                return out

            def mk_idx(ptr_i16_2d, tag):
                idx = spool.tile([128, FB], I16, tag=tag)
                nc.vector.tensor_tensor(idx[:], ptr_i16_2d, offs_c[:], ALU.add)
                return idx

            def mk_idxp(ptr_i16_2d, tag):
                idxp = spool.tile([128, FB, 2], I16, tag=tag)
                nc.vector.scalar_tensor_tensor(
                    idxp[:], bc_last(ptr_i16_2d, 2), 2.0, offs_p2[:],
                    ALU.mult, ALU.add)
                return idxp

            # ---------------- load ----------------
            tok_e = {}
            tok_n = {}
            for b in range(B_LOC):
                flat = tok_d[b].rearrange("w k c -> (w k c)")
                for c in range(NCH):
                    te = tokpool.tile([128, K * 9], F32, tag=f"te{b}{c}")
                    nc.sync.dma_start(
                        out=te[:],
                        in_=flat[c * 128 * 576:(c + 1) * 128 * 576]
                        .rearrange("(p f) -> p f", p=128))
                    tok_e[b, c] = te
                    tn = tokpool.tile([128, K * 9], F32, tag=f"tn{b}{c}")
                    if c < NCH - 1:
                        nc.sync.dma_start(
                            out=tn[:],
                            in_=flat[(c * 128 + 1) * 576:(c * 128 + 129) * 576]
                            .rearrange("(p f) -> p f", p=128))
                    else:
                        nc.vector.memset(tn[:], 0)
                        nc.sync.dma_start(
                            out=tn[0:127, :],
                            in_=flat[(c * 128 + 1) * 576:(c * 128 + 128) * 576]
                            .rearrange("(p f) -> p f", p=127))
                    tok_n[b, c] = tn

            def col_e(b, c, j):
                return tok_e[b, c].rearrange("p (k c) -> p k c", c=9)[:, :, j]

            def col_n(b, c, j):
                return tok_n[b, c].rearrange("p (k c) -> p k c", c=9)[:, :, j]

            ssum = spool.tile([128, NCH, B_LOC, K], F32, tag="ssum0")
            for b in range(B_LOC):
                for c in range(NCH):
                    nc.scalar.activation(ssum[:, c, b, :], col_e(b, c, 0),
                                         ACT.Square)
            # ---------------- matching ----------------
            fwdf = spool.tile([128, NCH, B_LOC, K], F32, tag="fwdf")
            match_rc = {}

            def emit_cons(idx):
                b, c = divmod(idx, NCH)
                # prologue: Pool saturates while DVE idles -> give DVE the
                # subtractions for the first chunks
                sub_eng = nc.vector if idx < 2 else nc.gpsimd
                ra = kkpool.tile([128, K, K], F32, tag="ra")
                rc = kkpool.tile([128, K, K], F32, tag=f"rc{idx % 2}")
                i1 = kkpool.tile([128, K, K], I16, tag=f"i1{idx % 2}")
                sub_eng.tensor_tensor(ra[:], bc_last(col_e(b, c, 4)),
                                      bc_mid(col_n(b, c, 3)), ALU.subtract)
                nc.scalar.activation(ra[:], ra[:], ACT.Abs)
                sub_eng.tensor_tensor(rc[:], bc_last(col_e(b, c, 6)),
                                      bc_mid(col_n(b, c, 5)), ALU.subtract)
                nc.scalar.activation(rc[:], rc[:], ACT.Square)
                nc.gpsimd.tensor_scalar(rc[:], rc[:], 0.25, None, ALU.is_gt)
                nc.gpsimd.tensor_tensor(rc[:], rc[:], ra[:], ALU.add)
                rp = kkpool.tile([128, K, K], F32, tag="rp")
                sub_eng.tensor_tensor(rp[:], bc_mid(col_n(b, c, 7)),
                                      bc_last(col_e(b, c, 8)), ALU.subtract)
                nc.scalar.activation(i1[:], rp[:], ACT.Copy, scale=INV2PI)
                nc.vector.scalar_tensor_tensor(rp[:], rp[:], INV2PI, i1[:],
                                               ALU.mult, ALU.subtract)
                nc.scalar.activation(rp[:], rp[:], ACT.Square)
                nc.vector.scalar_tensor_tensor(rc[:], rp[:], C2, rc[:],
                                               ALU.is_gt, ALU.add)
                match_rc[idx] = rc

            stage_d = spool.tile([128, NCH, B_LOC, K], I16, tag="stage_d")
            stage_i = spool.tile([128, NCH, B_LOC, K], I16, tag="stage_i")

            def emit_sel(idx):
                b, c = divmod(idx, NCH)
                rc = match_rc.pop(idx)
                i1 = kkpool.tile([128, K, K], I16, tag=f"i1{idx % 2}")
                i2 = kkpool.tile([128, K, K], I16, tag="i2")
                rowmin = kkpool.tile([128, K], F32, tag=f"rowmin{idx % 2}")
                colmin = kkpool.tile([128, K], F32, tag=f"colmin{idx % 2}")
                nc.vector.tensor_reduce(rowmin[:], rc[:], AX.X, ALU.min)
                nc.vector.tensor_reduce(colmin[:],
                                        rc.rearrange("p a b -> p b a"),
                                        AX.X, ALU.min)
                # column side: first argmin via iota_rev (exact tie-break)
                nc.vector.tensor_tensor(i1[:],
                                        rc.rearrange("p a b -> p b a"),
                                        bc_last(colmin[:]), ALU.is_equal)
                nc.vector.tensor_tensor(i1[:], i1[:], bc_mid(iota_rev[:]),
                                        ALU.mult)
                prvrev = kkpool.tile([128, K], I16, tag=f"prvrev{idx % 2}")
                nc.vector.tensor_reduce(prvrev[:], i1[:], AX.X, ALU.max)
                # row-side value check at col-min rows; first col-min row
                # must have its row minimum here
                nc.vector.tensor_tensor(i2[:],
                                        rc.rearrange("p a b -> p b a"),
                                        bc_mid(rowmin[:]), ALU.is_equal)
                nc.vector.tensor_tensor(i1[:], i1[:], i2[:], ALU.mult)
                amx = kkpool.tile([128, K], I16, tag=f"amx{idx % 2}")
                nc.vector.tensor_reduce(amx[:], i1[:], AX.X, ALU.max)
                medge = kkpool.tile([128, K], I16, tag=f"medge{idx % 2}")
                nc.vector.tensor_tensor(medge[:], amx[:], prvrev[:],
                                        ALU.is_equal)
                hvc = kkpool.tile([128, K], I16, tag=f"hvc{idx % 2}")
                nc.vector.tensor_scalar(hvc[:], colmin[:], 0.5, None,
                                        ALU.is_le)
                nc.vector.tensor_tensor(medge[:], medge[:], hvc[:], ALU.mult)
                # staging (reversed along k': duplicate fwdf targets resolve
                # to the smallest k' = reference argmin)
                iexp = kkpool.tile([128, K], I16, tag=f"iexp{idx % 2}")
                nc.vector.tensor_scalar(iexp[:], prvrev[:], -1.0,
                                        float(K + b * K + c * 128),
                                        ALU.mult, ALU.add)
                im = kkpool.tile([128, K], I16, tag=f"im{idx % 2}")
                nc.vector.scalar_tensor_tensor(im[:], medge[:], 16384.0,
                                               iexp[:], ALU.mult, ALU.add)
                nc.vector.tensor_scalar(im[:], im[:], 16384.0, None,
                                        ALU.subtract)
                if c == NCH - 1:
                    # window 511 has no successor: mask row p=127
                    nc.vector.scalar_tensor_tensor(
                        im[:], m127i[:], 16384.0, im[:], ALU.mult, ALU.add)
                    nc.vector.tensor_scalar(im[:], im[:], 16384.0, None,
                                            ALU.subtract)
                nc.vector.tensor_copy(stage_i[:, c, b, ::-1], im[:])
                dv = kkpool.tile([128, K], I16, tag=f"dv{idx % 2}")
                nc.vector.tensor_scalar(dv[:], iota_up[:], 1.0,
                                        BIAS + b * K + 1.0, ALU.mult, ALU.add)
                nc.vector.tensor_tensor(dv[:], dv[:], medge[:], ALU.mult)
                nc.vector.tensor_copy(stage_d[:, c, b, ::-1], dv[:])

            for idx in range(B_LOC * NCH):
                emit_cons(idx)
                if idx >= 1:
                    emit_sel(idx - 1)
            emit_sel(B_LOC * NCH - 1)

            # one global scatter builds fwdf (by-source) from column claims
            fwdf_i = sc16(stage_d, stage_i.rearrange("p c b k -> p (c b k)"),
                          "fwdf_i")
            nc.scalar.activation(fwdf.rearrange("p c b k -> p (c b k)"),
                                 fwdf_i.rearrange("p c b k -> p (c b k)"),
                                 ACT.Copy)
            # inverse map from the resolved fwdf (injective, no duplicates)
            idxA = mk_idx(fwdf_i.rearrange("p c b k -> p (c b k)"), "w2")
            invA = sc16(iota_inv.rearrange("p (c b k) -> p c b k",
                                           c=NCH, b=B_LOC), idxA, "invA")


            # ---------------- inverse + inv0 ----------------
            hn0 = spool.tile([128, NCH, B_LOC, K], F32, tag="hn0")
            nc.vector.tensor_scalar(hn0[:], fwdf[:], 0.0, None, ALU.is_gt)
            invA_f = spool.tile([128, NCH, B_LOC, K], F32, tag="w1")
            nc.scalar.activation(invA_f.rearrange("p c b k -> p (c b k)"),
                                 invA.rearrange("p c b k -> p (c b k)"),
                                 ACT.Copy)
            ps0 = pe_shift(invA_f, -1, "pa")
            inv0f = spool.tile([128, NCH, B_LOC, K], F32, tag="inv0f")
            nc.scalar.activation(inv0f.rearrange("p c b k -> p (c b k)"),
                                 ps0.rearrange("p c b k -> p (c b k)"),
                                 ACT.Copy)

            # ---------------- backward doubling ----------------
            ptrs = [fwdf]
            inv_cur = invA
            for j in range(NSTEP):
                d = 1 << j
                ps_p = pe_shift(ptrs[j], d, "pa")
                ps_s = pe_shift(ssum, d, "pb")
                sptr_i = spool.tile([128, FB], I16, tag="w0")
                nc.scalar.activation(sptr_i[:],
                                     ps_p.rearrange("p c b k -> p (c b k)"),
                                     ACT.Copy)
                sss = spool.tile([128, NCH, B_LOC, K], F32, tag="w1")
                nc.scalar.activation(sss.rearrange("p c b k -> p (c b k)"),
                                     ps_s.rearrange("p c b k -> p (c b k)"),
                                     ACT.Copy)
                idx_pay = mk_idx(inv_cur.rearrange("p c b k -> p (c b k)"),
                                 "w2")
                idxp_pay = mk_idxp(inv_cur.rearrange("p c b k -> p (c b k)"),
                                   "w3")
                g_ptr = sc16(sptr_i, idx_pay, "w4")
                g_ss = sc32(sss, idxp_pay, "w5")
                take = spool.tile([128, NCH, B_LOC, K], F32, tag="w6")
                nc.vector.tensor_scalar(take[:], ptrs[j][:], THR, None,
                                        ALU.is_gt)
                g_ptr_f = spool.tile([128, NCH, B_LOC, K], F32, tag="w7")
                nc.scalar.activation(
                    g_ptr_f.rearrange("p c b k -> p (c b k)"),
                    g_ptr.rearrange("p c b k -> p (c b k)"), ACT.Copy)
                gss2 = spool.tile([128, NCH, B_LOC, K], F32, tag="w8")
                nc.vector.tensor_tensor(gss2[:], g_ss[:], take[:], ALU.mult)
                ssum2 = spool.tile([128, NCH, B_LOC, K], F32,
                                   tag=f"ssum{(j + 1) % 2}")
                nc.vector.tensor_tensor(ssum2[:], ssum[:], gss2[:], ALU.add)
                ssum = ssum2
                if j < NSTEP - 1:
                    pnew = spool.tile([128, NCH, B_LOC, K], F32,
                                      tag=f"ptr{j + 1}")
                    nc.vector.tensor_tensor(pnew[:], g_ptr_f[:], take[:],
                                            ALU.mult)
                    ptrs.append(pnew)
                if j < NSTEP - 1:
                    idxI = mk_idx(sptr_i[:], "w9")
                    inv_cur = sc16(inv_cur, idxI, f"inv{(j + 1) % 2}")

            # ---------------- head ids ----------------
            q = spool.tile([128, NCH, B_LOC, K], F32, tag="w6")
            nc.vector.tensor_scalar(q[:], inv0f[:], THR, None, ALU.is_le)
            nc.vector.tensor_tensor(q[:], q[:], hn0[:], ALU.mult)
            rowq = spool.tile([128, NCH, B_LOC], F32, tag="rowq")
            nc.vector.tensor_reduce(rowq[:], q[:], AX.X, ALU.add)
            mm_ex = pspool.tile([128, NCH * B_LOC], F32, tag="ph0")
            nc.tensor.matmul(mm_ex[:], tri[:],
                             rowq.rearrange("p c b -> p (c b)"),
                             start=True, stop=True)
            tot = pspool.tile([128, NCH * B_LOC], F32, tag="ph1")
            nc.tensor.matmul(tot[:], ones128[:],
                             rowq.rearrange("p c b -> p (c b)"),
                             start=True, stop=True)
            tot_s = spool.tile([128, NCH, B_LOC], F32, tag="tot_s")
            nc.vector.tensor_copy(tot_s.rearrange("p c b -> p (c b)"),
                                  tot[:])
            incl = spool.tile([128, NCH + 1, B_LOC], F32, tag="incl")
            nc.vector.memset(incl[:, 0:1, :], 0)
            for b in range(B_LOC):
                nc.vector.tensor_tensor_scan(
                    incl[:, 1:, b], tot_s[:, :, b], zer64[:, 0:NCH], 0.0,
                    ALU.add, ALU.add)
                nc.sync.dma_start(out=cnt_d[0:1, b:b + 1],
                                  in_=incl[0:1, NCH:NCH + 1, b])
            base = spool.tile([128, NCH, B_LOC], F32, tag="base")
            nc.vector.tensor_tensor(base.rearrange("p c b -> p (c b)"),
                                    mm_ex[:],
                                    incl[:, 0:NCH, :]
                                    .rearrange("p c b -> p (c b)"), ALU.add)
            kincl = spool.tile([128, NCH, B_LOC, K], F32, tag="w5")
            for b in range(B_LOC):
                for c in range(NCH):
                    nc.vector.tensor_tensor_scan(
                        kincl[:, c, b, :], q[:, c, b, :], zer64[:], 0.0,
                        ALU.add, ALU.add)
            base_bc = base.rearrange("p c b -> p c b ()").to_broadcast(
                [128, NCH, B_LOC, K])
            nc.vector.tensor_tensor(kincl[:], kincl[:], base_bc, ALU.add)
            nc.vector.tensor_tensor(kincl[:], kincl[:], q[:], ALU.subtract)
            nc.vector.tensor_scalar(kincl[:], kincl[:], 1.0, None, ALU.add)
            vid = spool.tile([128, NCH, B_LOC, K], F32, tag="vid0")
            nc.vector.tensor_tensor(vid[:], kincl[:], q[:], ALU.mult)

            # ---------------- forward doubling ----------------
            vsn = ssum
            bwd = inv0f
            for j in range(NSTEP):
                d = 1 << j
                ps_f = pe_shift(ptrs[j], -d, "pa")
                ps_v = pe_shift(vid, -d, "pb")
                ps_n = pe_shift(vsn, -d, "pc")
                if j < NSTEP - 1:
                    ps_b = pe_shift(bwd, -d, "pd")
                sfj_i = spool.tile([128, FB], I16, tag="w0")
                nc.scalar.activation(sfj_i[:],
                                     ps_f.rearrange("p c b k -> p (c b k)"),
                                     ACT.Copy)
                svid_i = spool.tile([128, FB], I16, tag="w9")
                nc.scalar.activation(svid_i[:],
                                     ps_v.rearrange("p c b k -> p (c b k)"),
                                     ACT.Copy)
                if j < NSTEP - 1:
                    sbw_i = spool.tile([128, FB], I16, tag="wA")
                    nc.scalar.activation(
                        sbw_i[:], ps_b.rearrange("p c b k -> p (c b k)"),
                        ACT.Copy)
                svsn = spool.tile([128, NCH, B_LOC, K], F32, tag="w1")
                nc.scalar.activation(svsn.rearrange("p c b k -> p (c b k)"),
                                     ps_n.rearrange("p c b k -> p (c b k)"),
                                     ACT.Copy)
                idx_f = mk_idx(sfj_i[:], "w2")
                idxp_f = mk_idxp(sfj_i[:], "w3")
                g_vid = sc16(svid_i.rearrange("p (c b k) -> p c b k",
                                              c=NCH, b=B_LOC), idx_f, "w4")
                if j < NSTEP - 1:
                    g_bw = sc16(sbw_i.rearrange("p (c b k) -> p c b k",
                                                c=NCH, b=B_LOC), idx_f, "wB")
                g_sn = sc32(svsn, idxp_f, "w5")
                take = spool.tile([128, NCH, B_LOC, K], F32, tag="w6")
                nc.vector.tensor_scalar(take[:], bwd[:], THR, None, ALU.is_gt)
                nt = spool.tile([128, NCH, B_LOC, K], F32, tag="w8")
                nc.vector.tensor_scalar(nt[:], take[:], -1.0, 1.0, ALU.mult,
                                        ALU.add)
                g_vid_f = spool.tile([128, NCH, B_LOC, K], F32, tag="w7")
                nc.scalar.activation(
                    g_vid_f.rearrange("p c b k -> p (c b k)"),
                    g_vid.rearrange("p c b k -> p (c b k)"), ACT.Copy)
                vid2 = spool.tile([128, NCH, B_LOC, K], F32,
                                  tag=f"vid{(j + 1) % 2}")
                nc.vector.tensor_tensor(vid2[:], vid[:], nt[:], ALU.mult)
                nc.vector.tensor_tensor(vid2[:], vid2[:], g_vid_f[:], ALU.add)
                vid = vid2
                vsn2 = spool.tile([128, NCH, B_LOC, K], F32,
                                  tag=f"vsn{(j + 1) % 2}")
                nc.vector.tensor_tensor(vsn2[:], vsn[:], nt[:], ALU.mult)
                nc.vector.tensor_tensor(vsn2[:], vsn2[:], g_sn[:], ALU.add)
                vsn = vsn2
                if j < NSTEP - 1:
                    bwd2 = spool.tile([128, NCH, B_LOC, K], F32,
                                      tag=f"bwd{(j + 1) % 2}")
                    nc.scalar.activation(
                        bwd2.rearrange("p c b k -> p (c b k)"),
                        g_bw.rearrange("p c b k -> p (c b k)"), ACT.Copy)
                    bwd = bwd2

            # ---------------- smoothing ----------------
            assigned = spool.tile([128, NCH, B_LOC, K], F32, tag="w6")
            nc.vector.tensor_scalar(assigned[:], vid[:], 0.0, None, ALU.is_gt)
            edge = spool.tile([128, NCH, B_LOC, K], F32, tag="w8")
            nc.vector.tensor_tensor(edge[:], hn0[:], assigned[:], ALU.mult)
            em = spool.tile([128, NCH, B_LOC, K], F32, tag="w1")
            nc.vector.tensor_tensor(em[:], fwdf[:], edge[:], ALU.mult)
            asg16 = spool.tile([128, NCH, B_LOC, K], I16, tag="asg16")
            nc.scalar.activation(asg16.rearrange("p c b k -> p (c b k)"),
                                 assigned.rearrange("p c b k -> p (c b k)"),
                                 ACT.Copy)
            edge16 = spool.tile([128, NCH, B_LOC, K], I16, tag="edge16")
            nc.scalar.activation(edge16.rearrange("p c b k -> p (c b k)"),
                                 edge.rearrange("p c b k -> p (c b k)"),
                                 ACT.Copy)
            em_i = spool.tile([128, FB], I16, tag="w0")
            nc.scalar.activation(em_i[:],
                                 em.rearrange("p c b k -> p (c b k)"),
                                 ACT.Copy)
            # col0 sqrt
            m0 = spool.tile([128, NCH, B_LOC, K], F32, tag="w7")
            nc.vector.tensor_scalar(m0[:], vsn[:], 0.0, None, ALU.is_gt)
            t0 = spool.tile([128, NCH, B_LOC, K], F32, tag="w8")
            nc.vector.tensor_tensor(t0[:], vsn[:], m0[:], ALU.mult)
            nc.vector.tensor_scalar(m0[:], m0[:], -1.0, 1.0, ALU.mult,
                                    ALU.add)
            nc.vector.tensor_tensor(t0[:], t0[:], m0[:], ALU.add)
            s0 = spool.tile([128, NCH, B_LOC, K], F32, tag="hn0")
            nc.scalar.activation(s0.rearrange("p c b k -> p (c b k)"),
                                 t0.rearrange("p c b k -> p (c b k)"),
                                 ACT.Sqrt)
            # gather col_n f/A/p at nxt (deliver to predecessor slot)
            idxp_inv = mk_idxp(invA.rearrange("p c b k -> p (c b k)"),
                               "w3")
            fN = {}
            fE = {}
            for j_src, nm in ((3, "f"), (5, "A"), (7, "p")):
                t = spool.tile([128, NCH, B_LOC, K], F32, tag={'f': 'w1', 'A': 'vid0', 'p': 'w6'}[nm])
                for b in range(B_LOC):
                    for c in range(NCH):
                        nc.scalar.activation(t[:, c, b, :], col_n(b, c, j_src),
                                             ACT.Copy)
                fN[nm] = t
            for j_src, nm in ((4, "f"), (6, "A"), (8, "p")):
                t = spool.tile([128, NCH, B_LOC, K], F32, tag={'f': 'bwd0', 'A': 'bwd1', 'p': 'vsn0'}[nm])
                for b in range(B_LOC):
                    for c in range(NCH):
                        nc.scalar.activation(t[:, c, b, :], col_e(b, c, j_src),
                                             ACT.Copy)
                fE[nm] = t
            f_g = sc32(fN["f"], idxp_inv, "w4")
            A_g = sc32(fN["A"], idxp_inv, "w7")
            p_g = sc32(fN["p"], idxp_inv, "w8")
            favg = spool.tile([128, NCH, B_LOC, K], F32, tag="ssum0")
            nc.vector.tensor_tensor(favg[:], fE["f"][:], f_g[:], ALU.add)
            nc.vector.tensor_scalar(favg[:], favg[:], 0.5, None, ALU.mult)
            Aavg = spool.tile([128, NCH, B_LOC, K], F32, tag="ssum1")
            nc.vector.tensor_tensor(Aavg[:], fE["A"][:], A_g[:], ALU.add)
            nc.vector.tensor_scalar(Aavg[:], Aavg[:], 0.5, None, ALU.mult)
            half = spool.tile([128, NCH, B_LOC, K], F32, tag="ptr1")
            nc.vector.tensor_tensor(half[:], p_g[:], fE["p"][:], ALU.subtract)
            nc.vector.tensor_scalar(half[:], half[:], INV2PI, None, ALU.mult)
            hr16 = spool.tile([128, FB], I16, tag="w9")
            nc.vector.tensor_copy(hr16[:],
                                  half.rearrange("p c b k -> p (c b k)"))
            hrf = spool.tile([128, NCH, B_LOC, K], F32, tag="bwd0")
            nc.scalar.activation(hrf.rearrange("p c b k -> p (c b k)"),
                                 hr16[:], ACT.Copy)
            nc.vector.tensor_tensor(half[:], half[:], hrf[:], ALU.subtract)
            nc.vector.tensor_scalar(half[:], half[:], PI, None, ALU.mult)
            p7v = spool.tile([128, NCH, B_LOC, K], F32, tag="bwd1")
            nc.vector.tensor_tensor(p7v[:], p_g[:], half[:], ALU.subtract)
            # scatter smoothed values to successor slots + shift -1
            idx_em = mk_idx(em_i[:], "w2")
            idxp_em = mk_idxp(em_i[:], "w3")
            s3 = sc32(favg, idxp_em, "vid0")
            s5 = sc32(Aavg, idxp_em, "w6")
            s7 = sc32(p7v, idxp_em, "vsn0")
            flg = sc16(ones_i16.rearrange("p (c b k) -> p c b k",
                                          c=NCH, b=B_LOC), idx_em, "wA")
            flg_f = spool.tile([128, NCH, B_LOC, K], F32, tag="w4")
            nc.scalar.activation(flg_f.rearrange("p c b k -> p (c b k)"),
                                 flg.rearrange("p c b k -> p (c b k)"),
                                 ACT.Copy)
            sh = {}
            for nm, t in (("3", s3), ("5", s5), ("7", s7)):
                psx = pe_shift(t, -1, {"3": "pa", "5": "pb", "7": "pc"}[nm])
                o = spool.tile([128, NCH, B_LOC, K], F32,
                               tag={"3": "ptr2", "5": "ptr3", "7": "ptr4"}[nm])
                nc.scalar.activation(o.rearrange("p c b k -> p (c b k)"),
                                     psx.rearrange("p c b k -> p (c b k)"),
                                     ACT.Copy)
                sh[nm] = o
            psxf = pe_shift(flg_f, -1, "pd")
            shf16 = spool.tile([128, NCH, B_LOC, K], I16, tag="shf16")
            nc.scalar.activation(shf16.rearrange("p c b k -> p (c b k)"),
                                 psxf.rearrange("p c b k -> p (c b k)"),
                                 ACT.Copy)

            # ---------------- assembly ----------------
            for b in range(B_LOC):
                for c in range(NCH):
                    ot = tokpool.tile([128, K * 10], F32, tag=f"ot{(b * NCH + c) % 2}")
                    ov = ot.rearrange("p (k c) -> p k c", c=10)
                    nc.scalar.activation(
                        ov[:, :, 0:9],
                        tok_e[b, c].rearrange("p (k c) -> p k c", c=9),
                        ACT.Copy)
                    nc.vector.copy_predicated(ov[:, :, 0], asg16[:, c, b, :],
                                              s0[:, c, b, :])
                    nc.vector.copy_predicated(ov[:, :, 3], shf16[:, c, b, :],
                                              sh["3"][:, c, b, :])
                    nc.vector.copy_predicated(ov[:, :, 4], edge16[:, c, b, :],
                                              favg[:, c, b, :])
                    nc.vector.copy_predicated(ov[:, :, 5], shf16[:, c, b, :],
                                              sh["5"][:, c, b, :])
                    nc.vector.copy_predicated(ov[:, :, 6], edge16[:, c, b, :],
                                              Aavg[:, c, b, :])
                    nc.vector.copy_predicated(ov[:, :, 7], shf16[:, c, b, :],
                                              sh["7"][:, c, b, :])
                    p8 = kkpool.tile([128, K], F32, tag="p8")
                    nc.vector.tensor_tensor(p8[:], col_e(b, c, 8),
                                            half[:, c, b, :], ALU.add)
                    nc.vector.copy_predicated(ov[:, :, 8], edge16[:, c, b, :],
                                              p8[:])
                    nc.vector.tensor_scalar(ov[:, :, 9], vid[:, c, b, :], 1.0,
                                            None, ALU.subtract)
                    nc.sync.dma_start(out=out_d[b, c * 128:(c + 1) * 128],
                                      in_=ot.rearrange("p (k c) -> p k c",
                                                       c=10))
    nc.compile()
    return nc


def kernel(tokens: np.ndarray) -> np.ndarray:
    tokens = np.ascontiguousarray(tokens, dtype=np.float32)
    if "nc" not in _CACHE:
        _CACHE["nc"] = build_kernel()
    nc = _CACHE["nc"]
    n_cores = 8
    in_maps = [{"tokens": tokens[2 * i:2 * i + 2]} for i in range(n_cores)]
    res = run_bass_kernel_spmd(nc, in_maps, list(range(n_cores)))
    outs = [res.results[i]["out"] for i in range(n_cores)]
    cnts = np.concatenate([res.results[i]["counts"].reshape(-1)
                           for i in range(n_cores)])
    out = np.concatenate(outs, axis=0)
    offs = np.concatenate([[0.0], np.cumsum(cnts)[:-1]]).astype(np.float32)
    c9 = out[..., 9]
    out[..., 9] = np.where(c9 >= 0, c9 + offs[:, None, None], c9)
    return out


if __name__ == "__main__":
    out = kernel(np.zeros((16, 512, 64, 9), np.float32))
    print("ok", out.shape)



# revision 6
# speedup vs baseline: 1.0161x; 1.0161x over previous
"""ChirpLinker Trainium2 Bass kernel (v4).

Exact-f32 matching, engine-balanced for TRN2:
- cons: 3 f32 subs on gpsimd; Act squares + round; two-scalar
  TensorScalar threshold tests (2x modes) + i16 penalty combine;
  rc = |d| overwritten to BIG via copy_predicated where invalid.
- sel: rowmin/colmin reduces; is_equal claims; i16 argmin max-trees
  (2x) instead of full reduces; batched staging with pre-reversed
  iota constants + one global scatter (duplicates resolve to the
  smallest k' = reference argmin).
- pointer-doubling scan phases with head-id work interleaved into the
  backward loop; smoothing/assembly as before.
All matching decisions are bit-identical to the f32 reference on the
fixed key-0 dataset (device rel err ~2e-10).
"""
import numpy as np

import concourse.bass as bass
import concourse.bacc as bacc_mod
import concourse.mybir as mybir
from concourse.bass_utils import run_bass_kernel_spmd
from concourse.tile import TileContext

F32 = mybir.dt.float32
F32R = mybir.dt.float32r
I16 = mybir.dt.int16
U16 = mybir.dt.uint16
ALU = mybir.AluOpType
AX = mybir.AxisListType
ACT = mybir.ActivationFunctionType

PI = float(np.float32(np.pi))
INV2PI = float(np.float32(1.0 / (2.0 * np.pi)))
THRW = float(np.float32(np.float32(0.5) - np.float32(INV2PI)))
C2 = float(np.float32(INV2PI) * np.float32(INV2PI))

B_LOC = 2
W = 512
K = 64
NCH = 4
NSTEP = 5
FB = NCH * B_LOC * K          # 512 flat scan free size
BIAS = 8192.0
THR = 8000.0                  # biased-null threshold

_CACHE = {}

# engine assignment knobs: 'v' = DVE, 'p' = Pool(gpsimd)
CFG = {
    'a': 'p', 'rp': 'p', 'd': 'p', 'rc': 'v', 'ec': 'v', 'er': 'v',
    't1a': 'v', 't1b': 'v', 'ta': 'v', 'w': 'v', 'tw': 'v',
    'pen': 'v', 'i1': 'v', 'i2': 'v',
}


def bc_last(ap, n=K):
    return ap.to_broadcast(list(ap.shape) + [n])


def bc_mid(ap2d, n=K):
    s = ap2d.shape
    return ap2d.rearrange("p (o k) -> p o k", o=1).to_broadcast([s[0], n, s[1]])


def build_kernel():
    nc = bacc_mod.Bacc("TRN2", target_bir_lowering=False)
    def E(k):
        return nc.vector if CFG[k] == 'v' else nc.gpsimd
    tok_d = nc.declare_dram_parameter("tokens", [B_LOC, W, K, 9], F32,
                                      isOutput=False)
    out_d = nc.declare_dram_parameter("out", [B_LOC, W, K, 10], F32,
                                      isOutput=True)
    cnt_d = nc.declare_dram_parameter("counts", [1, B_LOC], F32, isOutput=True)

    with TileContext(nc) as tc:
        with (
            tc.tile_pool(name="const", bufs=1) as cpool,
            tc.tile_pool(name="kk", bufs=1) as kkpool,
            tc.tile_pool(name="tok", bufs=1) as tokpool,
            tc.tile_pool(name="sc", bufs=1) as spool,
            tc.tile_pool(name="ps", bufs=1, space="PSUM") as pspool,
        ):
            # ---------------- constants ----------------
            # reversed row-iota over the middle axis: val(a,b) = K - a
            iota_rm = cpool.tile([128, K, K], I16)
            nc.gpsimd.iota(iota_rm.rearrange("p a b -> p (a b)"),
                           pattern=[[-1, K], [0, K]], base=K,
                           channel_multiplier=0)
            big1 = cpool.tile([128, 1], F32)
            nc.vector.memset(big1[:], 4.0)
            m127_i = cpool.tile([128, 1], I16, tag="m127i")
            nc.gpsimd.iota(m127_i[:], pattern=[[0, 1]], base=0,
                           channel_multiplier=1)
            m127 = cpool.tile([128, 1], I16, tag="m127")
            nc.vector.tensor_scalar(m127[:], m127_i[:], 127.0, None,
                                    ALU.is_lt)


            def pe_shift(x_tile, dd, tag):
                """x shifted by dd windows -> PSUM tile [128, NCH, B, K]."""
                lo, hi = smat[dd]
                ps = pspool.tile([128, NCH, B_LOC, K], F32, tag=tag)
                pf = ps.rearrange("p c b k -> p (c b k)")
                xf = x_tile.rearrange("p c b k -> p (c b k)")
                cb = B_LOC * K
                nc.tensor.matmul(pf[:, :], lo[:],
                                 xf[:, :],
                                 start=True, stop=False, skip_group_check=True)
                if dd > 0:
                    nc.tensor.matmul(pf[:, 0:(NCH - 1) * cb],
                                     hi[:],
                                     xf[:, cb:NCH * cb],
                                     start=False, stop=True,
                                     skip_group_check=True)
                else:
                    nc.tensor.matmul(pf[:, cb:NCH * cb],
                                     hi[:],
                                     xf[:, 0:(NCH - 1) * cb],
                                     start=False, stop=True,
                                     skip_group_check=True)
                return ps

            def sc16(data_i16, idx_i16, tag):
                out = spool.tile([128, NCH, B_LOC, K], I16, tag=tag)
                nc.gpsimd.local_scatter(
                    out.rearrange("p c b k -> p (c b k)"),
                    data_i16.rearrange("p c b k -> p (c b k)")
                    if len(data_i16.shape) == 4 else data_i16[:],
                    idx_i16[:] if len(idx_i16.shape) == 2
                    else idx_i16.rearrange("p c b k -> p (c b k)"),
                    channels=128, num_elems=FB, num_idxs=FB)
                return out

            def sc32(data_f32, idxp_i16, tag):
                """scatter f32 payload via u16 pairs; zero-filled slots."""
                out = spool.tile([128, NCH, B_LOC, K], F32, tag=tag)
                nc.gpsimd.local_scatter(
                    out.bitcast(U16).rearrange("p c b k -> p (c b k)"),
                    data_f32.bitcast(U16).rearrange("p c b k -> p (c b k)")
                    if len(data_f32.shape) == 4
                    else data_f32.bitcast(U16)[:],
                    idxp_i16.rearrange("p f e -> p (f e)"),
                    channels=128, num_elems=2 * FB, num_idxs=2 * FB)
                return out

            def mk_idx(ptr_i16_2d, tag):
                idx = spool.tile([128, FB], I16, tag=tag)
                nc.vector.tensor_tensor(idx[:], ptr_i16_2d, offs_c[:], ALU.add)
                return idx

            def mk_idxp(ptr_i16_2d, tag):
                idxp = spool.tile([128, FB, 2], I16, tag=tag)
                nc.vector.scalar_tensor_tensor(
                    idxp[:], bc_last(ptr_i16_2d, 2), 2.0, offs_p2[:],
                    ALU.mult, ALU.add)
                return idxp

            # ---------------- load ----------------
            tok_e = {}
            tok_n = {}
            for b in range(B_LOC):
                flat = tok_d[b].rearrange("w k c -> (w k c)")
                for c in range(NCH):
                    te = tokpool.tile([128, K * 9], F32, tag=f"te{b}{c}")
                    nc.sync.dma_start(
                        out=te[:],
                        in_=flat[c * 128 * 576:(c + 1) * 128 * 576]
                        .rearrange("(p f) -> p f", p=128))
                    tok_e[b, c] = te
                    tn = tokpool.tile([128, K * 9], F32, tag=f"tn{b}{c}")
                    if c < NCH - 1:
                        nc.sync.dma_start(
                            out=tn[:],
                            in_=flat[(c * 128 + 1) * 576:(c * 128 + 129) * 576]
                            .rearrange("(p f) -> p f", p=128))
                    else:
                        nc.vector.memset(tn[:], 0)
                        nc.sync.dma_start(
                            out=tn[0:127, :],
                            in_=flat[(c * 128 + 1) * 576:(c * 128 + 128) * 576]
                            .rearrange("(p f) -> p f", p=127))
                    tok_n[b, c] = tn

            def col_e(b, c, j):
                return tok_e[b, c].rearrange("p (k c) -> p k c", c=9)[:, :, j]

            def col_n(b, c, j):
                return tok_n[b, c].rearrange("p (k c) -> p k c", c=9)[:, :, j]

            ssum = spool.tile([128, NCH, B_LOC, K], F32, tag="ssum0")
            for b in range(B_LOC):
                for c in range(NCH):
                    nc.scalar.activation(ssum[:, c, b, :], col_e(b, c, 0),
                                         ACT.Square)

            # ---------------- matching ----------------
            # per-idx staged results ([p, c, b, k]); cm/prv/amx reversed
            rm_all = spool.tile([128, NCH, B_LOC, K], F32, tag="rm_all")
            cm_rev = spool.tile([128, NCH, B_LOC, K], F32, tag="cm_rev")
            prv_rev = spool.tile([128, NCH, B_LOC, K], I16, tag="w9")
            amx_rev = spool.tile([128, NCH, B_LOC, K], I16, tag="wA")

            def emit_cons(idx):
                b, c = divmod(idx, NCH)
                a = kkpool.tile([128, K, K], F32, tag="a")
                rp = kkpool.tile([128, K, K], F32, tag="rp")
                E('a').tensor_tensor(a[:], bc_last(col_e(b, c, 6)),
                                        bc_mid(col_n(b, c, 5)), ALU.subtract)
                E('rp').tensor_tensor(rp[:], bc_mid(col_n(b, c, 7)),
                                        bc_last(col_e(b, c, 8)), ALU.subtract)
                aa = kkpool.tile([128, K, K], F32, tag="x")
                nc.scalar.activation(aa[:], a[:], ACT.Square)
                ta = kkpool.tile([128, K, K], I16, tag="ta")
                E('ta').tensor_scalar(ta[:], aa[:], 0.25, None, ALU.is_gt)
                i_ = kkpool.tile([128, K, K], I16, tag="tw")
                nc.scalar.activation(i_[:], rp[:], ACT.Copy, scale=INV2PI)
                w = kkpool.tile([128, K, K], F32, tag="x")
                E('w').scalar_tensor_tensor(w[:], rp[:], INV2PI, i_[:],
                                            ALU.mult, ALU.subtract)
                wsq = kkpool.tile([128, K, K], F32, tag="rp")
                nc.scalar.activation(wsq[:], w[:], ACT.Square)
                tw = kkpool.tile([128, K, K], I16, tag="tw")
                E('tw').tensor_scalar(tw[:], wsq[:], C2, None, ALU.is_gt)
                E('pen').tensor_tensor(ta[:], ta[:], tw[:], ALU.max)
                pen = ta
                d = kkpool.tile([128, K, K], F32, tag="a")
                E('d').tensor_tensor(d[:], bc_last(col_e(b, c, 4)),
                                     bc_mid(col_n(b, c, 3)), ALU.subtract)
                rc = kkpool.tile([128, K, K], F32, tag="x")
                nc.scalar.activation(rc[:], d[:], ACT.Abs)
                nc.vector.copy_predicated(
                    rc.rearrange("p a b -> p (a b)"),
                    pen.rearrange("p a b -> p (a b)"),
                    big1.to_broadcast([128, K * K]))
                return rc

            def tree_max(i2, out_slice, knob='t1a'):
                t1 = kkpool.tile([128, 32, K], I16, tag="tw")
                E(knob).tensor_tensor(t1[:], i2[:, 0:32, :], i2[:, 32:64, :],
                                      ALU.max)
                t2 = kkpool.tile([128, 16, K], I16, tag="t2")
                nc.vector.tensor_tensor(t2[:], t1[:, 0:16, :], t1[:, 16:32, :],
                                        ALU.max)
                t3 = kkpool.tile([128, 8, K], I16, tag="t3")
                nc.vector.tensor_tensor(t3[:], t2[:, 0:8, :], t2[:, 8:16, :],
                                        ALU.max)
                t4 = kkpool.tile([128, 4, K], I16, tag="t4")
                nc.vector.tensor_tensor(t4[:], t3[:, 0:4, :], t3[:, 4:8, :],
                                        ALU.max)
                nc.vector.tensor_reduce(out_slice,
                                        t4.rearrange("p a b -> p b a"),
                                        AX.X, ALU.max)

            def emit_sel(idx, rc):
                b, c = divmod(idx, NCH)
                nc.vector.tensor_reduce(cm_rev[:, c, b, ::-1],
                                        rc.rearrange("p a b -> p b a"),
                                        AX.X, ALU.min)
                nc.vector.tensor_reduce(rm_all[:, c, b, :], rc[:], AX.X,
                                        ALU.min)
                e_col = kkpool.tile([128, K, K], I16, tag="ec")
                nc.vector.tensor_tensor(e_col[:], rc[:],
                                        bc_mid(cm_rev[:, c, b, ::-1]),
                                        ALU.is_equal)
                nc.vector.tensor_tensor(e_col[:], e_col[:], iota_rm[:],
                                        ALU.mult)
                i1 = e_col
                tree_max(i1, prv_rev[:, c, b, ::-1], 't1a')
                e_row = kkpool.tile([128, K, K], I16, tag="ta")
                nc.vector.tensor_tensor(e_row[:], rc[:],
                                        bc_last(rm_all[:, c, b, :]),
                                        ALU.is_equal)
                nc.vector.tensor_tensor(i1[:], i1[:], e_row[:], ALU.mult)
                tree_max(i1, amx_rev[:, c, b, ::-1], 't1b')

            for idx in range(B_LOC * NCH):
                emit_sel(idx, emit_cons(idx))

            # idx offset for inverse map: c*128 - 8193  (idx = ptr + offs)
            iota_inv = cpool.tile([128, FB], I16)
            nc.gpsimd.iota(iota_inv[:], pattern=[[0, NCH], [K, B_LOC], [1, K]],
                           base=int(BIAS) + 1, channel_multiplier=0)
            offs_c = cpool.tile([128, FB], I16)
            nc.gpsimd.iota(offs_c[:], pattern=[[B_LOC * K, NCH], [0, B_LOC],
                                               [0, K]],
                           base=-(int(BIAS) + 1), channel_multiplier=0)
            # u16-pair idx offset: 2*(c*128 - 8193) + e
            offs_p2 = cpool.tile([128, FB, 2], I16)
            nc.gpsimd.iota(offs_p2.rearrange("p f e -> p (f e)"),
                           pattern=[[2 * B_LOC * K, NCH], [0, B_LOC], [0, K],
                                    [1, 2]],
                           base=-2 * (int(BIAS) + 1), channel_multiplier=0)
            ones_i16 = cpool.tile([128, FB], I16)
            nc.vector.memset(ones_i16[:], 1)
            zer64 = cpool.tile([128, K], F32)
            nc.vector.memset(zer64[:], 0)
            tri_i = cpool.tile([128, 128], I16)
            nc.gpsimd.iota(tri_i[:], pattern=[[1, 128]], base=0,
                           channel_multiplier=-1)
            tri = cpool.tile([128, 128], F32)
            nc.vector.tensor_scalar(tri[:], tri_i[:], 0.0, None, ALU.is_gt)
            ones128 = cpool.tile([128, 128], F32)
            nc.vector.memset(ones128[:], 1.0)
            # claim-scatter constants (pre-reversed along k'):
            # dv_rev[c,b,j] = BIAS + b*K + (K-1-j) + 1
            dv_rev = cpool.tile([128, NCH, B_LOC, K], I16)
            nc.gpsimd.iota(dv_rev.rearrange("p c b k -> p (c b k)"),
                           pattern=[[0, NCH], [K, B_LOC], [-1, K]],
                           base=int(BIAS) + K, channel_multiplier=0)
            # scb2[c,b,j] = c*B*K + b*K + K   (idx = scb2 - prvrev_rev)
            scb2 = cpool.tile([128, NCH, B_LOC, K], I16)
            nc.gpsimd.iota(scb2.rearrange("p c b k -> p (c b k)"),
                           pattern=[[B_LOC * K, NCH], [K, B_LOC], [0, K]],
                           base=K, channel_multiplier=0)
            # shifted-identity stationaries for PE window shifts
            smat = {}
            for d in (1, 2, 4, 8, 16):
                for sgn in (1, -1):
                    dd = d * sgn
                    lo_i = cpool.tile([128, 128], I16, tag="slo_i")
                    nc.gpsimd.iota(lo_i[:], pattern=[[-1, 128]], base=-dd,
                                   channel_multiplier=1)
                    lo = cpool.tile([128, 128], F32, tag=f"slo{dd}")
                    nc.vector.tensor_scalar(lo[:], lo_i[:], 0.0, None,
                                            ALU.is_equal)
                    hi_i = cpool.tile([128, 128], I16, tag="shi_i")
                    base = (128 - dd) if dd > 0 else (-dd - 128)
                    nc.gpsimd.iota(hi_i[:], pattern=[[-1, 128]], base=base,
                                   channel_multiplier=1)
                    hi = cpool.tile([128, 128], F32, tag=f"shi{dd}")
                    nc.vector.tensor_scalar(hi[:], hi_i[:], 0.0, None,
                                            ALU.is_equal)
                    smat[dd] = (lo, hi)
            # ---- batched staging + global scatter (baseline semantics) ----
            medge = spool.tile([128, NCH, B_LOC, K], I16, tag="w0")
            nc.vector.tensor_tensor(medge.rearrange("p c b k -> p (c b k)"),
                                    amx_rev.rearrange("p c b k -> p (c b k)"),
                                    prv_rev.rearrange("p c b k -> p (c b k)"),
                                    ALU.is_equal)
            v1 = spool.tile([128, NCH, B_LOC, K], I16, tag="w2")
            nc.vector.tensor_scalar(v1.rearrange("p c b k -> p (c b k)"),
                                    cm_rev.rearrange("p c b k -> p (c b k)"),
                                    0.5, None, ALU.is_le)
            nc.vector.tensor_tensor(medge.rearrange("p c b k -> p (c b k)"),
                                    medge.rearrange("p c b k -> p (c b k)"),
                                    v1.rearrange("p c b k -> p (c b k)"),
                                    ALU.mult)
            im = spool.tile([128, NCH, B_LOC, K], I16, tag="w2")
            nc.vector.tensor_tensor(im.rearrange("p c b k -> p (c b k)"),
                                    scb2.rearrange("p c b k -> p (c b k)"),
                                    prv_rev.rearrange("p c b k -> p (c b k)"),
                                    ALU.subtract)
            nc.vector.scalar_tensor_tensor(
                im.rearrange("p c b k -> p (c b k)"),
                medge.rearrange("p c b k -> p (c b k)"), 16384.0,
                im.rearrange("p c b k -> p (c b k)"), ALU.mult, ALU.add)
            nc.vector.tensor_scalar(im.rearrange("p c b k -> p (c b k)"),
                                    im.rearrange("p c b k -> p (c b k)"),
                                    16384.0, None, ALU.subtract)
            # window 511 (p=127, c=NCH-1) has no successor
            ims = im[:, NCH - 1, :, :].rearrange("p b k -> p (b k)")
            nc.vector.scalar_tensor_tensor(
                ims, m127.to_broadcast([128, B_LOC * K]), 16384.0, ims,
                ALU.mult, ALU.add)
            nc.vector.tensor_scalar(ims, ims, 16384.0, None, ALU.subtract)
            dv = spool.tile([128, NCH, B_LOC, K], I16, tag="w9")
            nc.vector.tensor_tensor(dv.rearrange("p c b k -> p (c b k)"),
                                    dv_rev.rearrange("p c b k -> p (c b k)"),
                                    medge.rearrange("p c b k -> p (c b k)"),
                                    ALU.mult)
            fwdf_i = sc16(dv, im, "wB")
            fwdf = spool.tile([128, NCH, B_LOC, K], F32, tag="fwdf")
            nc.scalar.activation(fwdf.rearrange("p c b k -> p (c b k)"),
                                 fwdf_i.rearrange("p c b k -> p (c b k)"),
                                 ACT.Copy)
            # inverse map from the resolved fwdf (injective, no duplicates)
            idxA = mk_idx(fwdf_i.rearrange("p c b k -> p (c b k)"), "w2")
            invA = sc16(iota_inv.rearrange("p (c b k) -> p c b k",
                                           c=NCH, b=B_LOC), idxA, "invA")

            # ---------------- inverse + inv0 ----------------
            hn0 = spool.tile([128, NCH, B_LOC, K], F32, tag="hn0")
            nc.vector.tensor_scalar(hn0[:], fwdf[:], 0.0, None, ALU.is_gt)
            invA_f = spool.tile([128, NCH, B_LOC, K], F32, tag="w1")
            nc.scalar.activation(invA_f.rearrange("p c b k -> p (c b k)"),
                                 invA.rearrange("p c b k -> p (c b k)"),
                                 ACT.Copy)
            ps0 = pe_shift(invA_f, -1, "pa")
            inv0f = spool.tile([128, NCH, B_LOC, K], F32, tag="inv0f")
            nc.scalar.activation(inv0f.rearrange("p c b k -> p (c b k)"),
                                 ps0.rearrange("p c b k -> p (c b k)"),
                                 ACT.Copy)

            # ---------------- backward doubling ----------------
            ptrs = [fwdf]
            inv_cur = invA
            head_emitted = [False, False]

            def emit_head1():
                q = spool.tile([128, NCH, B_LOC, K], F32, tag="rm_all")
                nc.vector.tensor_scalar(q[:], inv0f[:], THR, None, ALU.is_le)
                nc.vector.tensor_tensor(q[:], q[:], hn0[:], ALU.mult)
                rowq = spool.tile([128, NCH, B_LOC], F32, tag="rowq")
                nc.vector.tensor_reduce(rowq[:], q[:], AX.X, ALU.add)
                mm_ex = pspool.tile([128, NCH * B_LOC], F32, tag="ph0")
                nc.tensor.matmul(mm_ex[:], tri[:],
                                 rowq.rearrange("p c b -> p (c b)"),
                                 start=True, stop=True)
                tot = pspool.tile([128, NCH * B_LOC], F32, tag="ph1")
                nc.tensor.matmul(tot[:], ones128[:],
                                 rowq.rearrange("p c b -> p (c b)"),
                                 start=True, stop=True)
                tot_s = spool.tile([128, NCH, B_LOC], F32, tag="tot_s")
                nc.vector.tensor_copy(tot_s.rearrange("p c b -> p (c b)"),
                                      tot[:])
                return q, mm_ex, tot_s

            def emit_head2(q, mm_ex, tot_s):
                incl = spool.tile([128, NCH + 1, B_LOC], F32, tag="incl")
                nc.vector.memset(incl[:, 0:1, :], 0)
                for b in range(B_LOC):
                    nc.vector.tensor_tensor_scan(
                        incl[:, 1:, b], tot_s[:, :, b], zer64[:, 0:NCH], 0.0,
                        ALU.add, ALU.add)
                    nc.sync.dma_start(out=cnt_d[0:1, b:b + 1],
                                      in_=incl[0:1, NCH:NCH + 1, b])
                base = spool.tile([128, NCH, B_LOC], F32, tag="base")
                nc.vector.tensor_tensor(base.rearrange("p c b -> p (c b)"),
                                        mm_ex[:],
                                        incl[:, 0:NCH, :]
                                        .rearrange("p c b -> p (c b)"),
                                        ALU.add)
                kincl = spool.tile([128, NCH, B_LOC, K], F32, tag="cm_rev")
                for b in range(B_LOC):
                    for c in range(NCH):
                        nc.vector.tensor_tensor_scan(
                            kincl[:, c, b, :], q[:, c, b, :], zer64[:], 0.0,
                            ALU.add, ALU.add)
                base_bc = base.rearrange("p c b -> p c b ()").to_broadcast(
                    [128, NCH, B_LOC, K])
                nc.vector.tensor_tensor(kincl[:], kincl[:], base_bc, ALU.add)
                nc.vector.tensor_tensor(kincl[:], kincl[:], q[:],
                                        ALU.subtract)
                nc.vector.tensor_scalar(kincl[:], kincl[:], 1.0, None,
                                        ALU.add)
                vid = spool.tile([128, NCH, B_LOC, K], F32, tag="vid0")
                nc.vector.tensor_tensor(vid[:], kincl[:], q[:], ALU.mult)
                return vid

            head_state = {}
            for j in range(NSTEP):
                d = 1 << j
                ps_p = pe_shift(ptrs[j], d, "pa")
                ps_s = pe_shift(ssum, d, "pb")
                sptr_i = spool.tile([128, FB], I16, tag="w0")
                nc.scalar.activation(sptr_i[:],
                                     ps_p.rearrange("p c b k -> p (c b k)"),
                                     ACT.Copy)
                sss = spool.tile([128, NCH, B_LOC, K], F32, tag="w1")
                nc.scalar.activation(sss.rearrange("p c b k -> p (c b k)"),
                                     ps_s.rearrange("p c b k -> p (c b k)"),
                                     ACT.Copy)
                idx_pay = mk_idx(inv_cur.rearrange("p c b k -> p (c b k)"),
                                 "w2")
                idxp_pay = mk_idxp(inv_cur.rearrange("p c b k -> p (c b k)"),
                                   "w3")
                g_ptr = sc16(sptr_i, idx_pay, "w4")
                g_ss = sc32(sss, idxp_pay, "w5")
                take = spool.tile([128, NCH, B_LOC, K], F32, tag="w6")
                nc.vector.tensor_scalar(take[:], ptrs[j][:], THR, None,
                                        ALU.is_gt)
                g_ptr_f = spool.tile([128, NCH, B_LOC, K], F32, tag="w7")
                nc.scalar.activation(
                    g_ptr_f.rearrange("p c b k -> p (c b k)"),
                    g_ptr.rearrange("p c b k -> p (c b k)"), ACT.Copy)
                gss2 = spool.tile([128, NCH, B_LOC, K], F32, tag="w8")
                nc.vector.tensor_tensor(gss2[:], g_ss[:], take[:], ALU.mult)
                ssum2 = spool.tile([128, NCH, B_LOC, K], F32,
                                   tag=f"ssum{(j + 1) % 2}")
                nc.vector.tensor_tensor(ssum2[:], ssum[:], gss2[:], ALU.add)
                ssum = ssum2
                if j < NSTEP - 1:
                    pnew = spool.tile([128, NCH, B_LOC, K], F32,
                                      tag=f"ptr{j + 1}")
                    nc.vector.tensor_tensor(pnew[:], g_ptr_f[:], take[:],
                                            ALU.mult)
                    ptrs.append(pnew)
                if j < NSTEP - 1:
                    idxI = mk_idx(sptr_i[:], "w9")
                    inv_cur = sc16(inv_cur, idxI, f"inv{(j + 1) % 2}")
                if j == 0:
                    head_state['h1'] = emit_head1()
                elif j == 1:
                    head_state['vid'] = emit_head2(*head_state['h1'])

            # ---------------- head ids (interleaved above) ----------------
            vid = head_state['vid']

            # ---------------- forward doubling ----------------
            vsn = ssum
            bwd = inv0f
            for j in range(NSTEP):
                d = 1 << j
                ps_f = pe_shift(ptrs[j], -d, "pa")
                ps_v = pe_shift(vid, -d, "pb")
                ps_n = pe_shift(vsn, -d, "pc")
                if j < NSTEP - 1:
                    ps_b = pe_shift(bwd, -d, "pd")
                sfj_i = spool.tile([128, FB], I16, tag="w0")
                nc.scalar.activation(sfj_i[:],
                                     ps_f.rearrange("p c b k -> p (c b k)"),
                                     ACT.Copy)
                svid_i = spool.tile([128, FB], I16, tag="w9")
                nc.scalar.activation(svid_i[:],
                                     ps_v.rearrange("p c b k -> p (c b k)"),
                                     ACT.Copy)
                if j < NSTEP - 1:
                    sbw_i = spool.tile([128, FB], I16, tag="wA")
                    nc.scalar.activation(
                        sbw_i[:], ps_b.rearrange("p c b k -> p (c b k)"),
                        ACT.Copy)
                svsn = spool.tile([128, NCH, B_LOC, K], F32, tag="w1")
                nc.scalar.activation(svsn.rearrange("p c b k -> p (c b k)"),
                                     ps_n.rearrange("p c b k -> p (c b k)"),
                                     ACT.Copy)
                idx_f = mk_idx(sfj_i[:], "w2")
                idxp_f = mk_idxp(sfj_i[:], "w3")
                g_vid = sc16(svid_i.rearrange("p (c b k) -> p c b k",
                                              c=NCH, b=B_LOC), idx_f, "w4")
                if j < NSTEP - 1:
                    g_bw = sc16(sbw_i.rearrange("p (c b k) -> p c b k",
                                                c=NCH, b=B_LOC), idx_f, "wB")
                g_sn = sc32(svsn, idxp_f, "w5")
                take = spool.tile([128, NCH, B_LOC, K], F32, tag="w6")
                nc.vector.tensor_scalar(take[:], bwd[:], THR, None, ALU.is_gt)
                nt = spool.tile([128, NCH, B_LOC, K], F32, tag="w8")
                nc.vector.tensor_scalar(nt[:], take[:], -1.0, 1.0, ALU.mult,
                                        ALU.add)
                g_vid_f = spool.tile([128, NCH, B_LOC, K], F32, tag="w7")
                nc.scalar.activation(
                    g_vid_f.rearrange("p c b k -> p (c b k)"),
                    g_vid.rearrange("p c b k -> p (c b k)"), ACT.Copy)
                vid2 = spool.tile([128, NCH, B_LOC, K], F32,
                                  tag=f"vid{(j + 1) % 2}")
                nc.vector.tensor_tensor(vid2[:], vid[:], nt[:], ALU.mult)
                nc.vector.tensor_tensor(vid2[:], vid2[:], g_vid_f[:], ALU.add)
                vid = vid2
                vsn2 = spool.tile([128, NCH, B_LOC, K], F32,
                                  tag=f"vsn{(j + 1) % 2}")
                nc.vector.tensor_tensor(vsn2[:], vsn[:], nt[:], ALU.mult)
                nc.vector.tensor_tensor(vsn2[:], vsn2[:], g_sn[:], ALU.add)
                vsn = vsn2
                if j < NSTEP - 1:
                    bwd2 = spool.tile([128, NCH, B_LOC, K], F32,
                                      tag=f"bwd{(j + 1) % 2}")
                    nc.scalar.activation(
                        bwd2.rearrange("p c b k -> p (c b k)"),
                        g_bw.rearrange("p c b k -> p (c b k)"), ACT.Copy)
                    bwd = bwd2

            # ---------------- smoothing ----------------
            assigned = spool.tile([128, NCH, B_LOC, K], F32, tag="w6")
            nc.vector.tensor_scalar(assigned[:], vid[:], 0.0, None, ALU.is_gt)
            edge = spool.tile([128, NCH, B_LOC, K], F32, tag="w8")
            nc.vector.tensor_tensor(edge[:], hn0[:], assigned[:], ALU.mult)
            em = spool.tile([128, NCH, B_LOC, K], F32, tag="w1")
            nc.vector.tensor_tensor(em[:], fwdf[:], edge[:], ALU.mult)
            asg16 = spool.tile([128, NCH, B_LOC, K], I16, tag="asg16")
            nc.scalar.activation(asg16.rearrange("p c b k -> p (c b k)"),
                                 assigned.rearrange("p c b k -> p (c b k)"),
                                 ACT.Copy)
            edge16 = spool.tile([128, NCH, B_LOC, K], I16, tag="edge16")
            nc.scalar.activation(edge16.rearrange("p c b k -> p (c b k)"),
                                 edge.rearrange("p c b k -> p (c b k)"),
                                 ACT.Copy)
            em_i = spool.tile([128, FB], I16, tag="w0")
            nc.scalar.activation(em_i[:],
                                 em.rearrange("p c b k -> p (c b k)"),
                                 ACT.Copy)
            # col0 sqrt
            m0 = spool.tile([128, NCH, B_LOC, K], F32, tag="w7")
            nc.vector.tensor_scalar(m0[:], vsn[:], 0.0, None, ALU.is_gt)
            t0 = spool.tile([128, NCH, B_LOC, K], F32, tag="w8")
            nc.vector.tensor_tensor(t0[:], vsn[:], m0[:], ALU.mult)
            nc.vector.tensor_scalar(m0[:], m0[:], -1.0, 1.0, ALU.mult,
                                    ALU.add)
            nc.vector.tensor_tensor(t0[:], t0[:], m0[:], ALU.add)
            s0 = spool.tile([128, NCH, B_LOC, K], F32, tag="hn0")
            nc.scalar.activation(s0.rearrange("p c b k -> p (c b k)"),
                                 t0.rearrange("p c b k -> p (c b k)"),
                                 ACT.Sqrt)
            # gather col_n f/A/p at nxt (deliver to predecessor slot)
            idxp_inv = mk_idxp(invA.rearrange("p c b k -> p (c b k)"),
                               "w3")
            fN = {}
            fE = {}
            for j_src, nm in ((3, "f"), (5, "A"), (7, "p")):
                t = spool.tile([128, NCH, B_LOC, K], F32,
                               tag={'f': 'w1', 'A': 'vid0', 'p': 'w6'}[nm])
                for b in range(B_LOC):
                    for c in range(NCH):
                        nc.scalar.activation(t[:, c, b, :], col_n(b, c, j_src),
                                             ACT.Copy)
                fN[nm] = t
            for j_src, nm in ((4, "f"), (6, "A"), (8, "p")):
                t = spool.tile([128, NCH, B_LOC, K], F32,
                               tag={'f': 'bwd0', 'A': 'bwd1', 'p': 'vsn0'}[nm])
                for b in range(B_LOC):
                    for c in range(NCH):
                        nc.scalar.activation(t[:, c, b, :], col_e(b, c, j_src),
                                             ACT.Copy)
                fE[nm] = t
            f_g = sc32(fN["f"], idxp_inv, "w4")
            A_g = sc32(fN["A"], idxp_inv, "w7")
            p_g = sc32(fN["p"], idxp_inv, "w8")
            favg = spool.tile([128, NCH, B_LOC, K], F32, tag="ssum0")
            nc.vector.tensor_tensor(favg[:], fE["f"][:], f_g[:], ALU.add)
            nc.vector.tensor_scalar(favg[:], favg[:], 0.5, None, ALU.mult)
            Aavg = spool.tile([128, NCH, B_LOC, K], F32, tag="ssum1")
            nc.vector.tensor_tensor(Aavg[:], fE["A"][:], A_g[:], ALU.add)
            nc.vector.tensor_scalar(Aavg[:], Aavg[:], 0.5, None, ALU.mult)
            half = spool.tile([128, NCH, B_LOC, K], F32, tag="ptr1")
            nc.vector.tensor_tensor(half[:], p_g[:], fE["p"][:], ALU.subtract)
            nc.vector.tensor_scalar(half[:], half[:], INV2PI, None, ALU.mult)
            hr16 = spool.tile([128, FB], I16, tag="w9")
            nc.vector.tensor_copy(hr16[:],
                                  half.rearrange("p c b k -> p (c b k)"))
            hrf = spool.tile([128, NCH, B_LOC, K], F32, tag="bwd0")
            nc.scalar.activation(hrf.rearrange("p c b k -> p (c b k)"),
                                 hr16[:], ACT.Copy)
            nc.vector.tensor_tensor(half[:], half[:], hrf[:], ALU.subtract)
            nc.vector.tensor_scalar(half[:], half[:], PI, None, ALU.mult)
            p7v = spool.tile([128, NCH, B_LOC, K], F32, tag="bwd1")
            nc.vector.tensor_tensor(p7v[:], p_g[:], half[:], ALU.subtract)
            # scatter smoothed values to successor slots + shift -1
            idx_em = mk_idx(em_i[:], "w2")
            idxp_em = mk_idxp(em_i[:], "w3")
            s3 = sc32(favg, idxp_em, "vid0")
            s5 = sc32(Aavg, idxp_em, "w6")
            s7 = sc32(p7v, idxp_em, "vsn0")
            flg = sc16(ones_i16.rearrange("p (c b k) -> p c b k",
                                          c=NCH, b=B_LOC), idx_em, "wA")
            flg_f = spool.tile([128, NCH, B_LOC, K], F32, tag="w4")
            nc.scalar.activation(flg_f.rearrange("p c b k -> p (c b k)"),
                                 flg.rearrange("p c b k -> p (c b k)"),
                                 ACT.Copy)
            sh = {}
            for nm, t in (("3", s3), ("5", s5), ("7", s7)):
                psx = pe_shift(t, -1, {"3": "pa", "5": "pb", "7": "pc"}[nm])
                o = spool.tile([128, NCH, B_LOC, K], F32,
                               tag={"3": "ptr2", "5": "ptr3", "7": "ptr4"}[nm])
                nc.scalar.activation(o.rearrange("p c b k -> p (c b k)"),
                                     psx.rearrange("p c b k -> p (c b k)"),
                                     ACT.Copy)
                sh[nm] = o
            psxf = pe_shift(flg_f, -1, "pd")
            shf16 = spool.tile([128, NCH, B_LOC, K], I16, tag="shf16")
            nc.scalar.activation(shf16.rearrange("p c b k -> p (c b k)"),
                                 psxf.rearrange("p c b k -> p (c b k)"),
                                 ACT.Copy)

            # ---------------- assembly ----------------
            for b in range(B_LOC):
                for c in range(NCH):
                    ot = tokpool.tile([128, K * 10], F32,
                                      tag=f"ot{(b * NCH + c) % 2}")
                    ov = ot.rearrange("p (k c) -> p k c", c=10)
                    nc.scalar.activation(
                        ov[:, :, 0:9],
                        tok_e[b, c].rearrange("p (k c) -> p k c", c=9),
                        ACT.Copy)
                    nc.vector.copy_predicated(ov[:, :, 0], asg16[:, c, b, :],
                                              s0[:, c, b, :])
                    nc.vector.copy_predicated(ov[:, :, 3], shf16[:, c, b, :],
                                              sh["3"][:, c, b, :])
                    nc.vector.copy_predicated(ov[:, :, 4], edge16[:, c, b, :],
                                              favg[:, c, b, :])
                    nc.vector.copy_predicated(ov[:, :, 5], shf16[:, c, b, :],
                                              sh["5"][:, c, b, :])
                    nc.vector.copy_predicated(ov[:, :, 6], edge16[:, c, b, :],
                                              Aavg[:, c, b, :])
                    nc.vector.copy_predicated(ov[:, :, 7], shf16[:, c, b, :],
                                              sh["7"][:, c, b, :])
                    p8 = kkpool.tile([128, K], F32, tag="p8")
                    nc.vector.tensor_tensor(p8[:], col_e(b, c, 8),
                                            half[:, c, b, :], ALU.add)
                    nc.vector.copy_predicated(ov[:, :, 8], edge16[:, c, b, :],
                                              p8[:])
                    nc.vector.tensor_scalar(ov[:, :, 9], vid[:, c, b, :], 1.0,
                                            None, ALU.subtract)
                    nc.sync.dma_start(out=out_d[b, c * 128:(c + 1) * 128],
                                      in_=ot.rearrange("p (k c) -> p k c",
                                                       c=10))
    nc.compile()
    return nc


def kernel(tokens: np.ndarray) -> np.ndarray:
    tokens = np.ascontiguousarray(tokens, dtype=np.float32)
    if "nc" not in _CACHE:
        _CACHE["nc"] = build_kernel()
    nc = _CACHE["nc"]
    n_cores = 8
    in_maps = [{"tokens": tokens[2 * i:2 * i + 2]} for i in range(n_cores)]
    res = run_bass_kernel_spmd(nc, in_maps, list(range(n_cores)))
    outs = [res.results[i]["out"] for i in range(n_cores)]
    cnts = np.concatenate([res.results[i]["counts"].reshape(-1)
                           for i in range(n_cores)])
    out = np.concatenate(outs, axis=0)
    offs = np.concatenate([[0.0], np.cumsum(cnts)[:-1]]).astype(np.float32)
    c9 = out[..., 9]
    out[..., 9] = np.where(c9 >= 0, c9 + offs[:, None, None], c9)
    return out


if __name__ == "__main__":
    out = kernel(np.zeros((16, 512, 64, 9), np.float32))
    print("ok", out.shape)


# revision 8
# speedup vs baseline: 1.0352x; 1.0188x over previous
"""ChirpLinker Trainium2 Bass kernel (v4).

Exact-f32 matching, engine-balanced for TRN2:
- cons: 3 f32 subs on gpsimd; Act squares + round; two-scalar
  TensorScalar threshold tests (2x modes) + i16 penalty combine;
  rc = |d| overwritten to BIG via copy_predicated where invalid.
- sel: rowmin/colmin reduces; is_equal claims; i16 argmin max-trees
  (2x) instead of full reduces; batched staging with pre-reversed
  iota constants + one global scatter (duplicates resolve to the
  smallest k' = reference argmin).
- pointer-doubling scan phases with head-id work interleaved into the
  backward loop; smoothing/assembly as before.
All matching decisions are bit-identical to the f32 reference on the
fixed key-0 dataset (device rel err ~2e-10).
"""
import numpy as np

import concourse.bass as bass
import concourse.bacc as bacc_mod
import concourse.mybir as mybir
from concourse.bass_utils import run_bass_kernel_spmd
from concourse.tile import TileContext

F32 = mybir.dt.float32
F32R = mybir.dt.float32r
I16 = mybir.dt.int16
U16 = mybir.dt.uint16
ALU = mybir.AluOpType
AX = mybir.AxisListType
ACT = mybir.ActivationFunctionType

PI = float(np.float32(np.pi))
INV2PI = float(np.float32(1.0 / (2.0 * np.pi)))
THRW = float(np.float32(np.float32(0.5) - np.float32(INV2PI)))
C2 = float(np.float32(INV2PI) * np.float32(INV2PI))

B_LOC = 2
W = 512
K = 64
NCH = 4
NSTEP = 5
FB = NCH * B_LOC * K          # 512 flat scan free size
BIAS = 8192.0
THR = 8000.0                  # biased-null threshold

_CACHE = {}

# engine assignment knobs: 'v' = DVE, 'p' = Pool(gpsimd)
CFG = {
    'a': 'p', 'rp': 'p', 'd': 'p', 'rc': 'v', 'ec': 'v', 'er': 'v',
    't1a': 'v', 't1b': 'v', 'ta': 'v', 'w': 'v', 'tw': 'v',
    'pen': 'v', 'i1': 'v', 'i2': 'v',
}


def bc_last(ap, n=K):
    return ap.to_broadcast(list(ap.shape) + [n])


def bc_mid(ap2d, n=K):
    s = ap2d.shape
    return ap2d.rearrange("p (o k) -> p o k", o=1).to_broadcast([s[0], n, s[1]])


def build_kernel():
    nc = bacc_mod.Bacc("TRN2", target_bir_lowering=False)
    def E(k):
        return nc.vector if CFG[k] == 'v' else nc.gpsimd
    tok_d = nc.declare_dram_parameter("tokens", [B_LOC, W, K, 9], F32,
                                      isOutput=False)
    out_d = nc.declare_dram_parameter("out", [B_LOC, W, K, 10], F32,
                                      isOutput=True)
    cnt_d = nc.declare_dram_parameter("counts", [1, B_LOC], F32, isOutput=True)

    with TileContext(nc) as tc:
        with (
            tc.tile_pool(name="const", bufs=1) as cpool,
            tc.tile_pool(name="kk", bufs=1) as kkpool,
            tc.tile_pool(name="tok", bufs=1) as tokpool,
            tc.tile_pool(name="sc", bufs=1) as spool,
            tc.tile_pool(name="ps", bufs=1, space="PSUM") as pspool,
        ):
            # ---------------- constants ----------------
            # reversed doubled row-iota over middle: val(a,b) = 2*(K - a)
            iota_rm = cpool.tile([128, K, K], I16)
            nc.gpsimd.iota(iota_rm.rearrange("p a b -> p (a b)"),
                           pattern=[[-2, K], [0, K]], base=2 * K,
                           channel_multiplier=0)
            big1 = cpool.tile([128, 1], F32)
            nc.vector.memset(big1[:], 4.0)
            m127_i = cpool.tile([128, 1], I16, tag="m127i")
            nc.gpsimd.iota(m127_i[:], pattern=[[0, 1]], base=0,
                           channel_multiplier=1)
            m127 = cpool.tile([128, 1], I16, tag="m127")
            nc.vector.tensor_scalar(m127[:], m127_i[:], 127.0, None,
                                    ALU.is_lt)


            def pe_shift(x_tile, dd, tag):
                """x shifted by dd windows -> PSUM tile [128, NCH, B, K]."""
                lo, hi = smat[dd]
                ps = pspool.tile([128, NCH, B_LOC, K], F32, tag=tag)
                pf = ps.rearrange("p c b k -> p (c b k)")
                xf = x_tile.rearrange("p c b k -> p (c b k)")
                cb = B_LOC * K
                nc.tensor.matmul(pf[:, :], lo[:],
                                 xf[:, :],
                                 start=True, stop=False, skip_group_check=True)
                if dd > 0:
                    nc.tensor.matmul(pf[:, 0:(NCH - 1) * cb],
                                     hi[:],
                                     xf[:, cb:NCH * cb],
                                     start=False, stop=True,
                                     skip_group_check=True)
                else:
                    nc.tensor.matmul(pf[:, cb:NCH * cb],
                                     hi[:],
                                     xf[:, 0:(NCH - 1) * cb],
                                     start=False, stop=True,
                                     skip_group_check=True)
                return ps

            def sc16(data_i16, idx_i16, tag):
                out = spool.tile([128, NCH, B_LOC, K], I16, tag=tag)
                nc.gpsimd.local_scatter(
                    out.rearrange("p c b k -> p (c b k)"),
                    data_i16.rearrange("p c b k -> p (c b k)")
                    if len(data_i16.shape) == 4 else data_i16[:],
                    idx_i16[:] if len(idx_i16.shape) == 2
                    else idx_i16.rearrange("p c b k -> p (c b k)"),
                    channels=128, num_elems=FB, num_idxs=FB)
                return out

            def sc32(data_f32, idxp_i16, tag):
                """scatter f32 payload via u16 pairs; zero-filled slots."""
                out = spool.tile([128, NCH, B_LOC, K], F32, tag=tag)
                nc.gpsimd.local_scatter(
                    out.bitcast(U16).rearrange("p c b k -> p (c b k)"),
                    data_f32.bitcast(U16).rearrange("p c b k -> p (c b k)")
                    if len(data_f32.shape) == 4
                    else data_f32.bitcast(U16)[:],
                    idxp_i16.rearrange("p f e -> p (f e)"),
                    channels=128, num_elems=2 * FB, num_idxs=2 * FB)
                return out

            def mk_idx(ptr_i16_2d, tag):
                idx = spool.tile([128, FB], I16, tag=tag)
                nc.vector.tensor_tensor(idx[:], ptr_i16_2d, offs_c[:], ALU.add)
                return idx

            def mk_idxp(ptr_i16_2d, tag):
                idxp = spool.tile([128, FB, 2], I16, tag=tag)
                nc.vector.scalar_tensor_tensor(
                    idxp[:], bc_last(ptr_i16_2d, 2), 2.0, offs_p2[:],
                    ALU.mult, ALU.add)
                return idxp

            # ---------------- load ----------------
            tok_e = {}
            tok_n = {}
            for b in range(B_LOC):
                flat = tok_d[b].rearrange("w k c -> (w k c)")
                for c in range(NCH):
                    te = tokpool.tile([128, K * 9], F32, tag=f"te{b}{c}")
                    nc.sync.dma_start(
                        out=te[:],
                        in_=flat[c * 128 * 576:(c + 1) * 128 * 576]
                        .rearrange("(p f) -> p f", p=128))
                    tok_e[b, c] = te
                    tn = tokpool.tile([128, K * 9], F32, tag=f"tn{b}{c}")
                    if c < NCH - 1:
                        nc.sync.dma_start(
                            out=tn[:],
                            in_=flat[(c * 128 + 1) * 576:(c * 128 + 129) * 576]
                            .rearrange("(p f) -> p f", p=128))
                    else:
                        nc.vector.memset(tn[:], 0)
                        nc.sync.dma_start(
                            out=tn[0:127, :],
                            in_=flat[(c * 128 + 1) * 576:(c * 128 + 128) * 576]
                            .rearrange("(p f) -> p f", p=127))
                    tok_n[b, c] = tn

            def col_e(b, c, j):
                return tok_e[b, c].rearrange("p (k c) -> p k c", c=9)[:, :, j]

            def col_n(b, c, j):
                return tok_n[b, c].rearrange("p (k c) -> p k c", c=9)[:, :, j]

            ssum = spool.tile([128, NCH, B_LOC, K], F32, tag="ssum0")
            for b in range(B_LOC):
                for c in range(NCH):
                    nc.scalar.activation(ssum[:, c, b, :], col_e(b, c, 0),
                                         ACT.Square)

            # ---------------- matching ----------------
            # per-idx staged results ([p, c, b, k]); cm/prv/amx reversed
            rm_all = spool.tile([128, NCH, B_LOC, K], F32, tag="rm_all")
            cm_rev = spool.tile([128, NCH, B_LOC, K], F32, tag="cm_rev")
            prv_rev = spool.tile([128, NCH, B_LOC, K], I16, tag="w9")

            def emit_cons(idx):
                b, c = divmod(idx, NCH)
                rp = kkpool.tile([128, K, K], F32, tag="rp")
                E('rp').tensor_tensor(rp[:], bc_mid(col_n(b, c, 7)),
                                        bc_last(col_e(b, c, 8)), ALU.subtract)
                i_ = kkpool.tile([128, K, K], I16, tag="tw")
                nc.vector.tensor_scalar(i_[:], rp[:], INV2PI, None, ALU.mult)
                a = kkpool.tile([128, K, K], F32, tag="a")
                E('a').tensor_tensor(a[:], bc_last(col_e(b, c, 6)),
                                        bc_mid(col_n(b, c, 5)), ALU.subtract)
                aa = kkpool.tile([128, K, K], F32, tag="x")
                nc.scalar.activation(aa[:], a[:], ACT.Square)
                ta = kkpool.tile([128, K, K], I16, tag="ta")
                E('ta').tensor_scalar(ta[:], aa[:], 0.25, None, ALU.is_gt)
                w = kkpool.tile([128, K, K], F32, tag="x")
                E('w').scalar_tensor_tensor(w[:], rp[:], INV2PI, i_[:],
                                            ALU.mult, ALU.subtract)
                wsq = kkpool.tile([128, K, K], F32, tag="rp")
                nc.scalar.activation(wsq[:], w[:], ACT.Square)
                tw = kkpool.tile([128, K, K], I16, tag="tw")
                E('tw').tensor_scalar(tw[:], wsq[:], C2, None, ALU.is_gt)
                E('pen').tensor_tensor(ta[:], ta[:], tw[:], ALU.max)
                pen = ta
                d = kkpool.tile([128, K, K], F32, tag="a")
                E('d').tensor_tensor(d[:], bc_last(col_e(b, c, 4)),
                                     bc_mid(col_n(b, c, 3)), ALU.subtract)
                rc = kkpool.tile([128, K, K], F32, tag="x")
                nc.scalar.activation(rc[:], d[:], ACT.Abs)
                nc.vector.copy_predicated(
                    rc.rearrange("p a b -> p (a b)"),
                    pen.rearrange("p a b -> p (a b)"),
                    big1.to_broadcast([128, K * K]))
                return rc

            def tree_max(i2, out_slice, knob='t1a'):
                t1 = kkpool.tile([128, 32, K], I16, tag="tw")
                E(knob).tensor_tensor(t1[:], i2[:, 0:32, :], i2[:, 32:64, :],
                                      ALU.max)
                t2 = kkpool.tile([128, 16, K], I16, tag="t2")
                nc.vector.tensor_tensor(t2[:], t1[:, 0:16, :], t1[:, 16:32, :],
                                        ALU.max)
                t3 = kkpool.tile([128, 8, K], I16, tag="t3")
                nc.vector.tensor_tensor(t3[:], t2[:, 0:8, :], t2[:, 8:16, :],
                                        ALU.max)
                t4 = kkpool.tile([128, 4, K], I16, tag="t4")
                nc.vector.tensor_tensor(t4[:], t3[:, 0:4, :], t3[:, 4:8, :],
                                        ALU.max)
                nc.vector.tensor_reduce(out_slice,
                                        t4.rearrange("p a b -> p b a"),
                                        AX.X, ALU.max)

            def emit_sel(idx, rc):
                b, c = divmod(idx, NCH)
                nc.vector.tensor_reduce(cm_rev[:, c, b, ::-1],
                                        rc.rearrange("p a b -> p b a"),
                                        AX.X, ALU.min)
                nc.vector.tensor_reduce(rm_all[:, c, b, :], rc[:], AX.X,
                                        ALU.min)
                e_row = kkpool.tile([128, K, K], I16, tag="ta")
                nc.vector.tensor_tensor(e_row[:], rc[:],
                                        bc_last(rm_all[:, c, b, :]),
                                        ALU.is_equal)
                nc.vector.tensor_tensor(e_row[:], e_row[:], iota_rm[:],
                                        ALU.add)
                e_col = kkpool.tile([128, K, K], I16, tag="ec")
                nc.vector.tensor_tensor(e_col[:], rc[:],
                                        bc_mid(cm_rev[:, c, b, ::-1]),
                                        ALU.is_equal)
                nc.vector.tensor_tensor(e_col[:], e_col[:], e_row[:],
                                        ALU.mult)
                i1 = e_col
                tree_max(i1, prv_rev[:, c, b, ::-1], 't1a')

            for idx in range(B_LOC * NCH):
                emit_sel(idx, emit_cons(idx))

            # idx offset for inverse map: c*128 - 8193  (idx = ptr + offs)
            iota_inv = cpool.tile([128, FB], I16)
            nc.gpsimd.iota(iota_inv[:], pattern=[[0, NCH], [K, B_LOC], [1, K]],
                           base=int(BIAS) + 1, channel_multiplier=0)
            offs_c = cpool.tile([128, FB], I16)
            nc.gpsimd.iota(offs_c[:], pattern=[[B_LOC * K, NCH], [0, B_LOC],
                                               [0, K]],
                           base=-(int(BIAS) + 1), channel_multiplier=0)
            # u16-pair idx offset: 2*(c*128 - 8193) + e
            offs_p2 = cpool.tile([128, FB, 2], I16)
            nc.gpsimd.iota(offs_p2.rearrange("p f e -> p (f e)"),
                           pattern=[[2 * B_LOC * K, NCH], [0, B_LOC], [0, K],
                                    [1, 2]],
                           base=-2 * (int(BIAS) + 1), channel_multiplier=0)
            ones_i16 = cpool.tile([128, FB], I16)
            nc.vector.memset(ones_i16[:], 1)
            zer64 = cpool.tile([128, K], F32)
            nc.vector.memset(zer64[:], 0)
            tri_i = cpool.tile([128, 128], I16)
            nc.gpsimd.iota(tri_i[:], pattern=[[1, 128]], base=0,
                           channel_multiplier=-1)
            tri = cpool.tile([128, 128], F32)
            nc.vector.tensor_scalar(tri[:], tri_i[:], 0.0, None, ALU.is_gt)
            ones128 = cpool.tile([128, 128], F32)
            nc.vector.memset(ones128[:], 1.0)
            # claim-scatter constants (pre-reversed along k'):
            # dv_rev[c,b,j] = BIAS + b*K + (K-1-j) + 1
            dv_rev = cpool.tile([128, NCH, B_LOC, K], I16)
            nc.gpsimd.iota(dv_rev.rearrange("p c b k -> p (c b k)"),
                           pattern=[[0, NCH], [K, B_LOC], [-1, K]],
                           base=int(BIAS) + K, channel_multiplier=0)
            # scb2[c,b,j] = c*B*K + b*K + K   (idx = scb2 - prvrev_rev)
            scb2 = cpool.tile([128, NCH, B_LOC, K], I16)
            nc.gpsimd.iota(scb2.rearrange("p c b k -> p (c b k)"),
                           pattern=[[B_LOC * K, NCH], [K, B_LOC], [0, K]],
                           base=K, channel_multiplier=0)
            # shifted-identity stationaries for PE window shifts
            smat = {}
            for d in (1, 2, 4, 8, 16):
                for sgn in (1, -1):
                    dd = d * sgn
                    lo_i = cpool.tile([128, 128], I16, tag="slo_i")
                    nc.gpsimd.iota(lo_i[:], pattern=[[-1, 128]], base=-dd,
                                   channel_multiplier=1)
                    lo = cpool.tile([128, 128], F32, tag=f"slo{dd}")
                    nc.vector.tensor_scalar(lo[:], lo_i[:], 0.0, None,
                                            ALU.is_equal)
                    hi_i = cpool.tile([128, 128], I16, tag="shi_i")
                    base = (128 - dd) if dd > 0 else (-dd - 128)
                    nc.gpsimd.iota(hi_i[:], pattern=[[-1, 128]], base=base,
                                   channel_multiplier=1)
                    hi = cpool.tile([128, 128], F32, tag=f"shi{dd}")
                    nc.vector.tensor_scalar(hi[:], hi_i[:], 0.0, None,
                                            ALU.is_equal)
                    smat[dd] = (lo, hi)
            # ---- batched staging + global scatter ----
            # prv_rev holds packed 2*prv + erbit; unpack via round trick
            prvp = spool.tile([128, NCH, B_LOC, K], I16, tag="w0")
            nc.vector.tensor_scalar(prvp.rearrange("p c b k -> p (c b k)"),
                                    prv_rev.rearrange("p c b k -> p (c b k)"),
                                    0.5, -0.25, ALU.mult, ALU.add)
            medge = spool.tile([128, NCH, B_LOC, K], I16, tag="w3")
            nc.vector.scalar_tensor_tensor(
                medge.rearrange("p c b k -> p (c b k)"),
                prvp.rearrange("p c b k -> p (c b k)"), -2.0,
                prv_rev.rearrange("p c b k -> p (c b k)"),
                ALU.mult, ALU.add)
            prv_rev = prvp
            v1 = spool.tile([128, NCH, B_LOC, K], I16, tag="w2")
            nc.vector.tensor_scalar(v1.rearrange("p c b k -> p (c b k)"),
                                    cm_rev.rearrange("p c b k -> p (c b k)"),
                                    0.5, None, ALU.is_le)
            nc.vector.tensor_tensor(medge.rearrange("p c b k -> p (c b k)"),
                                    medge.rearrange("p c b k -> p (c b k)"),
                                    v1.rearrange("p c b k -> p (c b k)"),
                                    ALU.mult)
            im = spool.tile([128, NCH, B_LOC, K], I16, tag="w2")
            nc.vector.tensor_tensor(im.rearrange("p c b k -> p (c b k)"),
                                    scb2.rearrange("p c b k -> p (c b k)"),
                                    prv_rev.rearrange("p c b k -> p (c b k)"),
                                    ALU.subtract)
            nc.vector.scalar_tensor_tensor(
                im.rearrange("p c b k -> p (c b k)"),
                medge.rearrange("p c b k -> p (c b k)"), 16384.0,
                im.rearrange("p c b k -> p (c b k)"), ALU.mult, ALU.add)
            nc.vector.tensor_scalar(im.rearrange("p c b k -> p (c b k)"),
                                    im.rearrange("p c b k -> p (c b k)"),
                                    16384.0, None, ALU.subtract)
            # window 511 (p=127, c=NCH-1) has no successor
            ims = im[:, NCH - 1, :, :].rearrange("p b k -> p (b k)")
            nc.vector.scalar_tensor_tensor(
                ims, m127.to_broadcast([128, B_LOC * K]), 16384.0, ims,
                ALU.mult, ALU.add)
            nc.vector.tensor_scalar(ims, ims, 16384.0, None, ALU.subtract)
            dv = spool.tile([128, NCH, B_LOC, K], I16, tag="w9")
            nc.vector.tensor_tensor(dv.rearrange("p c b k -> p (c b k)"),
                                    dv_rev.rearrange("p c b k -> p (c b k)"),
                                    medge.rearrange("p c b k -> p (c b k)"),
                                    ALU.mult)
            fwdf_i = sc16(dv, im, "wB")
            fwdf = spool.tile([128, NCH, B_LOC, K], F32, tag="fwdf")
            nc.scalar.activation(fwdf.rearrange("p c b k -> p (c b k)"),
                                 fwdf_i.rearrange("p c b k -> p (c b k)"),
                                 ACT.Copy)
            # inverse map from the resolved fwdf (injective, no duplicates)
            idxA = mk_idx(fwdf_i.rearrange("p c b k -> p (c b k)"), "w2")
            invA = sc16(iota_inv.rearrange("p (c b k) -> p c b k",
                                           c=NCH, b=B_LOC), idxA, "invA")

            # ---------------- inverse + inv0 ----------------
            hn0 = spool.tile([128, NCH, B_LOC, K], F32, tag="hn0")
            nc.vector.tensor_scalar(hn0[:], fwdf[:], 0.0, None, ALU.is_gt)
            invA_f = spool.tile([128, NCH, B_LOC, K], F32, tag="w1")
            nc.scalar.activation(invA_f.rearrange("p c b k -> p (c b k)"),
                                 invA.rearrange("p c b k -> p (c b k)"),
                                 ACT.Copy)
            ps0 = pe_shift(invA_f, -1, "pa")
            inv0f = spool.tile([128, NCH, B_LOC, K], F32, tag="inv0f")
            nc.scalar.activation(inv0f.rearrange("p c b k -> p (c b k)"),
                                 ps0.rearrange("p c b k -> p (c b k)"),
                                 ACT.Copy)

            # ---------------- backward doubling ----------------
            ptrs = [fwdf]
            inv_cur = invA
            head_emitted = [False, False]

            def emit_head1():
                q = spool.tile([128, NCH, B_LOC, K], F32, tag="rm_all")
                nc.vector.tensor_scalar(q[:], inv0f[:], THR, None, ALU.is_le)
                nc.vector.tensor_tensor(q[:], q[:], hn0[:], ALU.mult)
                rowq = spool.tile([128, NCH, B_LOC], F32, tag="rowq")
                nc.vector.tensor_reduce(rowq[:], q[:], AX.X, ALU.add)
                mm_ex = pspool.tile([128, NCH * B_LOC], F32, tag="ph0")
                nc.tensor.matmul(mm_ex[:], tri[:],
                                 rowq.rearrange("p c b -> p (c b)"),
                                 start=True, stop=True)
                tot = pspool.tile([128, NCH * B_LOC], F32, tag="ph1")
                nc.tensor.matmul(tot[:], ones128[:],
                                 rowq.rearrange("p c b -> p (c b)"),
                                 start=True, stop=True)
                tot_s = spool.tile([128, NCH, B_LOC], F32, tag="tot_s")
                nc.vector.tensor_copy(tot_s.rearrange("p c b -> p (c b)"),
                                      tot[:])
                return q, mm_ex, tot_s

            def emit_head2(q, mm_ex, tot_s):
                incl = spool.tile([128, NCH + 1, B_LOC], F32, tag="incl")
                nc.vector.memset(incl[:, 0:1, :], 0)
                for b in range(B_LOC):
                    nc.vector.tensor_tensor_scan(
                        incl[:, 1:, b], tot_s[:, :, b], zer64[:, 0:NCH], 0.0,
                        ALU.add, ALU.add)
                    nc.sync.dma_start(out=cnt_d[0:1, b:b + 1],
                                      in_=incl[0:1, NCH:NCH + 1, b])
                base = spool.tile([128, NCH, B_LOC], F32, tag="base")
                nc.vector.tensor_tensor(base.rearrange("p c b -> p (c b)"),
                                        mm_ex[:],
                                        incl[:, 0:NCH, :]
                                        .rearrange("p c b -> p (c b)"),
                                        ALU.add)
                kincl = spool.tile([128, NCH, B_LOC, K], F32, tag="cm_rev")
                for b in range(B_LOC):
                    for c in range(NCH):
                        nc.vector.tensor_tensor_scan(
                            kincl[:, c, b, :], q[:, c, b, :], zer64[:], 0.0,
                            ALU.add, ALU.add)
                base_bc = base.rearrange("p c b -> p c b ()").to_broadcast(
                    [128, NCH, B_LOC, K])
                nc.vector.tensor_tensor(kincl[:], kincl[:], base_bc, ALU.add)
                nc.vector.tensor_tensor(kincl[:], kincl[:], q[:],
                                        ALU.subtract)
                nc.vector.tensor_scalar(kincl[:], kincl[:], 1.0, None,
                                        ALU.add)
                vid = spool.tile([128, NCH, B_LOC, K], F32, tag="vid0")
                nc.vector.tensor_tensor(vid[:], kincl[:], q[:], ALU.mult)
                return vid

            head_state = {}
            for j in range(NSTEP):
                d = 1 << j
                ps_p = pe_shift(ptrs[j], d, "pa")
                ps_s = pe_shift(ssum, d, "pb")
                sptr_i = spool.tile([128, FB], I16, tag="w0")
                nc.scalar.activation(sptr_i[:],
                                     ps_p.rearrange("p c b k -> p (c b k)"),
                                     ACT.Copy)
                sss = spool.tile([128, NCH, B_LOC, K], F32, tag="w1")
                nc.scalar.activation(sss.rearrange("p c b k -> p (c b k)"),
                                     ps_s.rearrange("p c b k -> p (c b k)"),
                                     ACT.Copy)
                idx_pay = mk_idx(inv_cur.rearrange("p c b k -> p (c b k)"),
                                 "w2")
                idxp_pay = mk_idxp(inv_cur.rearrange("p c b k -> p (c b k)"),
                                   "w3")
                g_ptr = sc16(sptr_i, idx_pay, "w4")
                g_ss = sc32(sss, idxp_pay, "w5")
                take = spool.tile([128, NCH, B_LOC, K], F32, tag="w6")
                nc.vector.tensor_scalar(take[:], ptrs[j][:], THR, None,
                                        ALU.is_gt)
                g_ptr_f = spool.tile([128, NCH, B_LOC, K], F32, tag="w7")
                nc.scalar.activation(
                    g_ptr_f.rearrange("p c b k -> p (c b k)"),
                    g_ptr.rearrange("p c b k -> p (c b k)"), ACT.Copy)
                gss2 = spool.tile([128, NCH, B_LOC, K], F32, tag="w8")
                nc.vector.tensor_tensor(gss2[:], g_ss[:], take[:], ALU.mult)
                ssum2 = spool.tile([128, NCH, B_LOC, K], F32,
                                   tag=f"ssum{(j + 1) % 2}")
                nc.vector.tensor_tensor(ssum2[:], ssum[:], gss2[:], ALU.add)
                ssum = ssum2
                if j < NSTEP - 1:
                    pnew = spool.tile([128, NCH, B_LOC, K], F32,
                                      tag=f"ptr{j + 1}")
                    nc.vector.tensor_tensor(pnew[:], g_ptr_f[:], take[:],
                                            ALU.mult)
                    ptrs.append(pnew)
                if j < NSTEP - 1:
                    idxI = mk_idx(sptr_i[:], "w9")
                    inv_cur = sc16(inv_cur, idxI, f"inv{(j + 1) % 2}")
                if j == 0:
                    head_state['h1'] = emit_head1()
                elif j == 1:
                    head_state['vid'] = emit_head2(*head_state['h1'])

            # ---------------- head ids (interleaved above) ----------------
            vid = head_state['vid']

            # ---------------- forward doubling ----------------
            vsn = ssum
            bwd = inv0f
            for j in range(NSTEP):
                d = 1 << j
                ps_f = pe_shift(ptrs[j], -d, "pa")
                ps_v = pe_shift(vid, -d, "pb")
                ps_n = pe_shift(vsn, -d, "pc")
                if j < NSTEP - 1:
                    ps_b = pe_shift(bwd, -d, "pd")
                sfj_i = spool.tile([128, FB], I16, tag="w0")
                nc.scalar.activation(sfj_i[:],
                                     ps_f.rearrange("p c b k -> p (c b k)"),
                                     ACT.Copy)
                svid_i = spool.tile([128, FB], I16, tag="w9")
                nc.scalar.activation(svid_i[:],
                                     ps_v.rearrange("p c b k -> p (c b k)"),
                                     ACT.Copy)
                if j < NSTEP - 1:
                    sbw_i = spool.tile([128, FB], I16, tag="wA")
                    nc.scalar.activation(
                        sbw_i[:], ps_b.rearrange("p c b k -> p (c b k)"),
                        ACT.Copy)
                svsn = spool.tile([128, NCH, B_LOC, K], F32, tag="w1")
                nc.scalar.activation(svsn.rearrange("p c b k -> p (c b k)"),
                                     ps_n.rearrange("p c b k -> p (c b k)"),
                                     ACT.Copy)
                idx_f = mk_idx(sfj_i[:], "w2")
                idxp_f = mk_idxp(sfj_i[:], "w3")
                g_vid = sc16(svid_i.rearrange("p (c b k) -> p c b k",
                                              c=NCH, b=B_LOC), idx_f, "w4")
                if j < NSTEP - 1:
                    g_bw = sc16(sbw_i.rearrange("p (c b k) -> p c b k",
                                                c=NCH, b=B_LOC), idx_f, "wB")
                g_sn = sc32(svsn, idxp_f, "w5")
                take = spool.tile([128, NCH, B_LOC, K], F32, tag="w6")
                nc.vector.tensor_scalar(take[:], bwd[:], THR, None, ALU.is_gt)
                nt = spool.tile([128, NCH, B_LOC, K], F32, tag="w8")
                nc.vector.tensor_scalar(nt[:], take[:], -1.0, 1.0, ALU.mult,
                                        ALU.add)
                g_vid_f = spool.tile([128, NCH, B_LOC, K], F32, tag="w7")
                nc.scalar.activation(
                    g_vid_f.rearrange("p c b k -> p (c b k)"),
                    g_vid.rearrange("p c b k -> p (c b k)"), ACT.Copy)
                vid2 = spool.tile([128, NCH, B_LOC, K], F32,
                                  tag=f"vid{(j + 1) % 2}")
                nc.vector.tensor_tensor(vid2[:], vid[:], nt[:], ALU.mult)
                nc.vector.tensor_tensor(vid2[:], vid2[:], g_vid_f[:], ALU.add)
                vid = vid2
                vsn2 = spool.tile([128, NCH, B_LOC, K], F32,
                                  tag=f"vsn{(j + 1) % 2}")
                nc.vector.tensor_tensor(vsn2[:], vsn[:], nt[:], ALU.mult)
                nc.vector.tensor_tensor(vsn2[:], vsn2[:], g_sn[:], ALU.add)
                vsn = vsn2
                if j < NSTEP - 1:
                    bwd2 = spool.tile([128, NCH, B_LOC, K], F32,
                                      tag=f"bwd{(j + 1) % 2}")
                    nc.scalar.activation(
                        bwd2.rearrange("p c b k -> p (c b k)"),
                        g_bw.rearrange("p c b k -> p (c b k)"), ACT.Copy)
                    bwd = bwd2

            # ---------------- smoothing ----------------
            assigned = spool.tile([128, NCH, B_LOC, K], F32, tag="w6")
            nc.vector.tensor_scalar(assigned[:], vid[:], 0.0, None, ALU.is_gt)
            edge = spool.tile([128, NCH, B_LOC, K], F32, tag="w8")
            nc.vector.tensor_tensor(edge[:], hn0[:], assigned[:], ALU.mult)
            em = spool.tile([128, NCH, B_LOC, K], F32, tag="w1")
            nc.vector.tensor_tensor(em[:], fwdf[:], edge[:], ALU.mult)
            asg16 = spool.tile([128, NCH, B_LOC, K], I16, tag="asg16")
            nc.scalar.activation(asg16.rearrange("p c b k -> p (c b k)"),
                                 assigned.rearrange("p c b k -> p (c b k)"),
                                 ACT.Copy)
            edge16 = spool.tile([128, NCH, B_LOC, K], I16, tag="edge16")
            nc.scalar.activation(edge16.rearrange("p c b k -> p (c b k)"),
                                 edge.rearrange("p c b k -> p (c b k)"),
                                 ACT.Copy)
            em_i = spool.tile([128, FB], I16, tag="w0")
            nc.scalar.activation(em_i[:],
                                 em.rearrange("p c b k -> p (c b k)"),
                                 ACT.Copy)
            # col0 sqrt
            m0 = spool.tile([128, NCH, B_LOC, K], F32, tag="w7")
            nc.vector.tensor_scalar(m0[:], vsn[:], 0.0, None, ALU.is_gt)
            t0 = spool.tile([128, NCH, B_LOC, K], F32, tag="w8")
            nc.vector.tensor_tensor(t0[:], vsn[:], m0[:], ALU.mult)
            nc.vector.tensor_scalar(m0[:], m0[:], -1.0, 1.0, ALU.mult,
                                    ALU.add)
            nc.vector.tensor_tensor(t0[:], t0[:], m0[:], ALU.add)
            s0 = spool.tile([128, NCH, B_LOC, K], F32, tag="hn0")
            nc.scalar.activation(s0.rearrange("p c b k -> p (c b k)"),
                                 t0.rearrange("p c b k -> p (c b k)"),
                                 ACT.Sqrt)
            # gather col_n f/A/p at nxt (deliver to predecessor slot)
            idxp_inv = mk_idxp(invA.rearrange("p c b k -> p (c b k)"),
                               "w3")
            fN = {}
            fE = {}
            for j_src, nm in ((3, "f"), (5, "A"), (7, "p")):
                t = spool.tile([128, NCH, B_LOC, K], F32,
                               tag={'f': 'w1', 'A': 'vid0', 'p': 'w6'}[nm])
                for b in range(B_LOC):
                    for c in range(NCH):
                        nc.scalar.activation(t[:, c, b, :], col_n(b, c, j_src),
                                             ACT.Copy)
                fN[nm] = t
            for j_src, nm in ((4, "f"), (6, "A"), (8, "p")):
                t = spool.tile([128, NCH, B_LOC, K], F32,
                               tag={'f': 'bwd0', 'A': 'bwd1', 'p': 'vsn0'}[nm])
                for b in range(B_LOC):
                    for c in range(NCH):
                        nc.scalar.activation(t[:, c, b, :], col_e(b, c, j_src),
                                             ACT.Copy)
                fE[nm] = t
            f_g = sc32(fN["f"], idxp_inv, "w4")
            A_g = sc32(fN["A"], idxp_inv, "w7")
            p_g = sc32(fN["p"], idxp_inv, "w8")
            favg = spool.tile([128, NCH, B_LOC, K], F32, tag="ssum0")
            nc.vector.tensor_tensor(favg[:], fE["f"][:], f_g[:], ALU.add)
            nc.vector.tensor_scalar(favg[:], favg[:], 0.5, None, ALU.mult)
            Aavg = spool.tile([128, NCH, B_LOC, K], F32, tag="ssum1")
            nc.vector.tensor_tensor(Aavg[:], fE["A"][:], A_g[:], ALU.add)
            nc.vector.tensor_scalar(Aavg[:], Aavg[:], 0.5, None, ALU.mult)
            half = spool.tile([128, NCH, B_LOC, K], F32, tag="ptr1")
            nc.vector.tensor_tensor(half[:], p_g[:], fE["p"][:], ALU.subtract)
            nc.vector.tensor_scalar(half[:], half[:], INV2PI, None, ALU.mult)
            hr16 = spool.tile([128, FB], I16, tag="w9")
            nc.vector.tensor_copy(hr16[:],
                                  half.rearrange("p c b k -> p (c b k)"))
            hrf = spool.tile([128, NCH, B_LOC, K], F32, tag="bwd0")
            nc.scalar.activation(hrf.rearrange("p c b k -> p (c b k)"),
                                 hr16[:], ACT.Copy)
            nc.vector.tensor_tensor(half[:], half[:], hrf[:], ALU.subtract)
            nc.vector.tensor_scalar(half[:], half[:], PI, None, ALU.mult)
            p7v = spool.tile([128, NCH, B_LOC, K], F32, tag="bwd1")
            nc.vector.tensor_tensor(p7v[:], p_g[:], half[:], ALU.subtract)
            # scatter smoothed values to successor slots + shift -1
            idx_em = mk_idx(em_i[:], "w2")
            idxp_em = mk_idxp(em_i[:], "w3")
            s3 = sc32(favg, idxp_em, "vid0")
            s5 = sc32(Aavg, idxp_em, "w6")
            s7 = sc32(p7v, idxp_em, "vsn0")
            flg = sc16(ones_i16.rearrange("p (c b k) -> p c b k",
                                          c=NCH, b=B_LOC), idx_em, "wA")
            flg_f = spool.tile([128, NCH, B_LOC, K], F32, tag="w4")
            nc.scalar.activation(flg_f.rearrange("p c b k -> p (c b k)"),
                                 flg.rearrange("p c b k -> p (c b k)"),
                                 ACT.Copy)
            sh = {}
            for nm, t in (("3", s3), ("5", s5), ("7", s7)):
                psx = pe_shift(t, -1, {"3": "pa", "5": "pb", "7": "pc"}[nm])
                o = spool.tile([128, NCH, B_LOC, K], F32,
                               tag={"3": "ptr2", "5": "ptr3", "7": "ptr4"}[nm])
                nc.scalar.activation(o.rearrange("p c b k -> p (c b k)"),
                                     psx.rearrange("p c b k -> p (c b k)"),
                                     ACT.Copy)
                sh[nm] = o
            psxf = pe_shift(flg_f, -1, "pd")
            shf16 = spool.tile([128, NCH, B_LOC, K], I16, tag="shf16")
            nc.scalar.activation(shf16.rearrange("p c b k -> p (c b k)"),
                                 psxf.rearrange("p c b k -> p (c b k)"),
                                 ACT.Copy)

            # ---------------- assembly ----------------
            for b in range(B_LOC):
                for c in range(NCH):
                    ot = tokpool.tile([128, K * 10], F32,
                                      tag=f"ot{(b * NCH + c) % 2}")
                    ov = ot.rearrange("p (k c) -> p k c", c=10)
                    nc.scalar.activation(
                        ov[:, :, 0:9],
                        tok_e[b, c].rearrange("p (k c) -> p k c", c=9),
                        ACT.Copy)
                    nc.vector.copy_predicated(ov[:, :, 0], asg16[:, c, b, :],
                                              s0[:, c, b, :])
                    nc.vector.copy_predicated(ov[:, :, 3], shf16[:, c, b, :],
                                              sh["3"][:, c, b, :])
                    nc.vector.copy_predicated(ov[:, :, 4], edge16[:, c, b, :],
                                              favg[:, c, b, :])
                    nc.vector.copy_predicated(ov[:, :, 5], shf16[:, c, b, :],
                                              sh["5"][:, c, b, :])
                    nc.vector.copy_predicated(ov[:, :, 6], edge16[:, c, b, :],
                                              Aavg[:, c, b, :])
                    nc.vector.copy_predicated(ov[:, :, 7], shf16[:, c, b, :],
                                              sh["7"][:, c, b, :])
                    p8 = kkpool.tile([128, K], F32, tag="p8")
                    nc.vector.tensor_tensor(p8[:], col_e(b, c, 8),
                                            half[:, c, b, :], ALU.add)
                    nc.vector.copy_predicated(ov[:, :, 8], edge16[:, c, b, :],
                                              p8[:])
                    nc.vector.tensor_scalar(ov[:, :, 9], vid[:, c, b, :], 1.0,
                                            None, ALU.subtract)
                    nc.sync.dma_start(out=out_d[b, c * 128:(c + 1) * 128],
                                      in_=ot.rearrange("p (k c) -> p k c",
                                                       c=10))
    nc.compile()
    return nc


def kernel(tokens: np.ndarray) -> np.ndarray:
    tokens = np.ascontiguousarray(tokens, dtype=np.float32)
    if "nc" not in _CACHE:
        _CACHE["nc"] = build_kernel()
    nc = _CACHE["nc"]
    n_cores = 8
    in_maps = [{"tokens": tokens[2 * i:2 * i + 2]} for i in range(n_cores)]
    res = run_bass_kernel_spmd(nc, in_maps, list(range(n_cores)))
    outs = [res.results[i]["out"] for i in range(n_cores)]
    cnts = np.concatenate([res.results[i]["counts"].reshape(-1)
                           for i in range(n_cores)])
    out = np.concatenate(outs, axis=0)
    offs = np.concatenate([[0.0], np.cumsum(cnts)[:-1]]).astype(np.float32)
    c9 = out[..., 9]
    out[..., 9] = np.where(c9 >= 0, c9 + offs[:, None, None], c9)
    return out


if __name__ == "__main__":
    out = kernel(np.zeros((16, 512, 64, 9), np.float32))
    print("ok", out.shape)
